# revision 56
# baseline (speedup 1.0000x reference)
"""Trainium2 Bass kernel for nn_MhaSelfAttenLayer (dense transformer layer).

Data-parallel over batch: each of the 8 NeuronCores runs the full layer on
one batch element. No collectives.

Precision plan (validated numerically; sim matches HW to ~1e-4):
- Attention path (QKV proj, V, out proj, attn*V) in fp8 e4m3 with DoubleRow
  matmuls; scores in bf16 (64-wide contraction gets no DoubleRow benefit);
  fp32 PSUM accumulation everywhere.
- FFN1 contracts 6 of 8 c-tiles in fp8 DoubleRow (x32/x8 scales) and the
  last 2 in bf16 (x16/x16, so both partials carry the same 256x PSUM
  scale, removed in the ReLU activation scale).  7/8 or 8/8 fp8 breaches
  the 2e-2 budget; 6/8 lands at 1.89e-2.  FFN2 stays bf16.
- Residual adds ride the PE: each out-proj / FFN2 PSUM accumulation group
  starts with identity.T @ residual (bf16), and the LayerNorms read stats
  straight from PSUM.  LN is scale-invariant, so the fp8 512x scale on the
  attention PSUM needs no unscale; x ships pre-scaled as bf16.
- Scale folding: qkT holds 256*q ; the 256^2 factor is removed inside the
  exp() activation scale (exact power of two), v/ctx/out-proj scales fold
  into existing copies, so fp8 costs no extra instructions.
"""

import math

import numpy as np
import ml_dtypes

import concourse.bass as bass
import concourse.tile as tile
from concourse import mybir
from concourse.bass_utils import run_bass_kernel_spmd
from concourse.vector_clock import ScopedClock, VectorClock

F32 = mybir.dt.float32
BF16 = mybir.dt.bfloat16
F8 = mybir.dt.float8e4
BF = ml_dtypes.bfloat16
E4 = ml_dtypes.float8_e4m3

N, T, C, H, HD, HID = 8, 1024, 1024, 16, 64, 4096
KT = C // 128           # 8 c-tiles
MT = T // 128           # 8 t-tiles
JT = HID // 128         # 32 hid-tiles
EPS = 1e-5
AF = mybir.ActivationFunctionType
OP = mybir.AluOpType
PM = mybir.MatmulPerfMode

SX = 8.0                # xq8 = SX*(x+pe)
SW = 32.0               # wqk8 = SW*w  -> qkT = 256*q
ESC = 1.0 / (256.0 * 256.0 * 8.0)   # exp arg unscale (2^-19), incl 1/sqrt(hd)
SE = 2.0                # ex = SE*exp(score)
SV = 16.0               # v8 = SV*v ; ctxT8 = SV*ctx
SWO = 32.0              # wo8 = SWO*wo
AOS = 1.0 / (SV * SWO)  # attn-out unscale

_patched = False


def _patch_drain():
    """This walrus build accepts at most 1 sem wait per instruction (2 for
    EventSemaphore). Tile's final drain packs every outstanding proc wait
    onto a single drain -> codegen error. Emit one drain per proc instead."""
    global _patched
    if _patched:
        return
    _patched = True

    def _split_drain_and_barrier(self, tick_clock, wait_clock):
        gclock = tick_clock.global_clock
        n = len(gclock)
        for proc in range(n):
            t = gclock[proc]
            if t <= 0:
                continue
            vc = VectorClock([0] * n)
            vc.require_at_least(proc, t)
            d = self.nc.sync.drain()
            wait_clock.add_sem_waits(d.ins, ScopedClock({None: vc}))
        self.nc.all_engine_barrier()
        popped = self.nc._tile_sem_poison_stack.pop()
        assert popped is self._sem_poison
        self.nc.clear_and_free_semaphores(list(self.sems.allocated().values()))
        self.nc.all_engine_barrier()

    tile.TileContext._drain_and_barrier = _split_drain_and_barrier


def _split_multiwait(nc):
    """This walrus build accepts at most one sem wait per instruction. Hoist
    excess waits onto freshly created same-engine nops placed immediately
    before the over-limit instruction (engine streams run in order, so the
    nop blocking first preserves the dependency)."""
    import bass_rust

    engmap = {
        mybir.EngineType.PE: nc.tensor,
        mybir.EngineType.DVE: nc.vector,
        mybir.EngineType.Activation: nc.scalar,
        mybir.EngineType.SP: nc.sync,
        mybir.EngineType.Pool: nc.gpsimd,
    }
    blocks = list(nc.main_func.blocks)
    records = []
    for bi, bb in enumerate(blocks):
        for ins in bb.instructions:
            si = ins.sync_info
            if si is None or not si.on_wait:
                continue
            waits = list(si.on_wait)
            limit = 2 if type(ins).__name__ == "InstEventSemaphore" else 1
            if len(waits) > limit:
                records.append((ins.name, ins, waits[:-limit]))
                si.on_wait = waits[-limit:]
    if not records:
        return
    carriers = {}
    nop_names = set()
    for name, ins, excess in records:
        lst = []
        for w in excess:
            nb = engmap[ins.engine].nop()
            nb.ins.sync_info = bass_rust.SyncInfo(on_wait=[w], on_update=[])
            nop_names.add(nb.ins.name)
            lst.append(nb.ins)
        carriers[name] = lst
    for bb in blocks:
        il = list(bb.instructions)
        out = []
        changed = False
        for ins in il:
            if ins.name in nop_names:
                changed = True
                continue
            if ins.name in carriers:
                out.extend(carriers[ins.name])
                changed = True
            out.append(ins)
        if changed:
            bb.instructions = out


def _host_constants():
    pos = np.arange(T, dtype=np.float32)[:, None]
    div = np.exp(
        np.arange(0, C, 2, dtype=np.float32) * (-math.log(10000.0) / C)
    )
    ang = pos * div
    pe = np.stack([np.sin(ang), np.cos(ang)], axis=-1).reshape(T, C)
    peT = np.ascontiguousarray(pe.T)                    # [C, T]

    ident = np.eye(128, dtype=np.float32)
    kk = np.arange(128)
    # mask01[k, q] = 0 where q < k (future key within diagonal block)
    mask01 = np.where(kk[None, :] < kk[:, None], 0.0, 1.0).astype(E4)
    return peT, ident, mask01


def _build(flags):
    """flags = (g1, b1ln, g2, b2ln, b1, b2) booleans for non-trivial params."""
    has_g1, has_b1ln, has_g2, has_b2ln, has_b1, has_b2 = flags
    _patch_drain()
    nc = bass.Bass(trn_type="TRN2")

    # ---- DRAM I/O ----
    xq_d = nc.dram_tensor("xq8", [128, KT, T], F8, kind="ExternalInput")
    x_tc = nc.dram_tensor("x_tc", [T, C], BF16, kind="ExternalInput")
    wqk_d = nc.dram_tensor("wqk8", [128, 16, KT, 128], F8, kind="ExternalInput")
    wv_d = nc.dram_tensor("wv8", [128, KT, C], F8, kind="ExternalInput")
    wo_d = nc.dram_tensor("wo8", [128, KT, C], F8, kind="ExternalInput")
    w18_d = nc.dram_tensor("w18", [128, 3, 2, JT, 128], F8,
                           kind="ExternalInput")
    w1t_d = nc.dram_tensor("w1t", [128, 2, JT, 128], BF16,
                           kind="ExternalInput")
    w2_d = nc.dram_tensor("w2b", [128, JT, C], BF16, kind="ExternalInput")
    idb_d = nc.dram_tensor("identb", [128, 128], BF16, kind="ExternalInput")
    mk_d = nc.dram_tensor("mask01", [128, 128], F8, kind="ExternalInput")
    if has_g1:
        g1_d = nc.dram_tensor("g1", [C], F32, kind="ExternalInput")
    if has_b1ln:
        b1ln_d = nc.dram_tensor("b1ln", [C], F32, kind="ExternalInput")
    if has_g2:
        g2_d = nc.dram_tensor("g2", [C], F32, kind="ExternalInput")
    if has_b2ln:
        b2ln_d = nc.dram_tensor("b2ln", [C], F32, kind="ExternalInput")
    if has_b1:
        b1_d = nc.dram_tensor("b1t", [128, JT], F32, kind="ExternalInput")
    if has_b2:
        b2_d = nc.dram_tensor("b2", [C], F32, kind="ExternalInput")
    out_d = nc.dram_tensor("out", [C, T], F32, kind="ExternalOutput")

    def bcast_ap(dram_1d, n):
        return bass.AP(tensor=dram_1d.tensor, offset=0, ap=[[0, 128], [1, n]])

    with tile.TileContext(nc) as tc:
        with (
            tc.tile_pool(name="consts", bufs=1) as consts,
            tc.tile_pool(name="smalls", bufs=12) as smalls,
            tc.tile_pool(name="p_hbf", bufs=1) as p_hbf,
            tc.tile_pool(name="p_hT", bufs=1) as p_hT,
        ):
            # ---- constants (ACT-ring DMAs; SP ring is kept clear for the
            # latency-critical xq8/wqk loads) ----
            zbias = consts.tile([128, 1], F32)
            nc.vector.memset(zbias, 0.0)
            nc.const_aps.aps[(F32, 0.0)] = zbias
            epsb = consts.tile([128, 1], F32)
            nc.vector.memset(epsb, EPS)
            lnb = consts.tile([128, 1], F32)
            nc.vector.memset(lnb, float(math.log(SE)))
            warm = consts.tile([128, 64], BF16)
            nc.vector.memset(warm, 0.25)
            tblw = smalls.tile([128, 1], F32, tag="tblw")
            nc.scalar.activation(tblw, epsb, AF.Exp, bias=0.0, scale=1.0)
            nc.scalar.activation(tblw, epsb, AF.Sqrt, bias=0.0, scale=1.0)
            mask01 = consts.tile([128, 128], F8)
            nc.gpsimd.dma_start(out=mask01, in_=mk_d[:, :])
            identb = consts.tile([128, 128], BF16)
            nc.gpsimd.dma_start(out=identb, in_=idb_d[:, :])
            g1bc = b1lnbc = g2bc = b2lnbc = b1sb = b2bc = None
            if has_g1:
                g1bc = consts.tile([128, C], F32)
                nc.scalar.dma_start(out=g1bc, in_=bcast_ap(g1_d, C))
            if has_b1ln:
                b1lnbc = consts.tile([128, C], F32)
                nc.scalar.dma_start(out=b1lnbc, in_=bcast_ap(b1ln_d, C))
            if has_g2:
                g2bc = consts.tile([128, C], F32)
                nc.scalar.dma_start(out=g2bc, in_=bcast_ap(g2_d, C))
            if has_b2ln:
                b2lnbc = consts.tile([128, C], F32)
                nc.scalar.dma_start(out=b2lnbc, in_=bcast_ap(b2ln_d, C))
            if has_b1:
                b1sb = consts.tile([128, JT], F32)
                nc.scalar.dma_start(out=b1sb, in_=b1_d[:, :])
            if has_b2:
                b2bc = consts.tile([128, C], F32)
                nc.scalar.dma_start(out=b2bc, in_=bcast_ap(b2_d, C))

            # ---- PE warm-up: ~6us of tiny matmuls during the input-DMA
            # wait so the HAM clock-gate reaches K=8/8 before real work ----
            with tc.tile_pool(name="pp_warm", bufs=1, space="PSUM") as pp_w:
                wps = pp_w.tile([64, 64], F32)
                for _ in range(84):
                    nc.tensor.matmul(wps, lhsT=warm[:, 0:64],
                                     rhs=warm[:, 0:64], start=True, stop=True)

            hbf = p_hbf.tile([128, MT, C], BF16)
            hT8 = p_hT.tile([128, 6, T], F8)
            hTb = p_hT.tile([128, 2, T], BF16)

            def layernorm(resid, out_tile, gbc, bbc, zpool):
                stats = smalls.tile([128, 2, 6], F32, tag="stats")
                nc.vector.bn_stats(out=stats[:, 0, :], in_=resid[:, 0:512])
                nc.vector.bn_stats(out=stats[:, 1, :], in_=resid[:, 512:1024])
                mv = smalls.tile([128, 2], F32, tag="mv")
                nc.vector.bn_aggr(out=mv, in_=stats)
                std = smalls.tile([128, 1], F32, tag="std")
                nc.scalar.activation(std, mv[:, 1:2], AF.Sqrt, bias=epsb)
                istd = smalls.tile([128, 1], F32, tag="istd")
                nc.vector.reciprocal(istd, std)
                nbias = smalls.tile([128, 1], F32, tag="nbias")
                nc.vector.tensor_scalar(
                    out=nbias, in0=mv[:, 0:1], scalar1=istd, scalar2=-1.0,
                    op0=OP.mult, op1=OP.mult,
                )
                if gbc is None and bbc is None:
                    # two halves: downstream per-128-col transposes unblock
                    # after half 0 (subtile deps), hiding the chain latency
                    nc.scalar.activation(
                        out_tile[:, 0:512], resid[:, 0:512],
                        AF.Identity, bias=nbias, scale=istd,
                    )
                    nc.scalar.activation(
                        out_tile[:, 512:1024], resid[:, 512:1024],
                        AF.Identity, bias=nbias, scale=istd,
                    )
                else:
                    z = zpool.tile([128, C], F32, tag="zln")
                    nc.vector.tensor_scalar(
                        out=z, in0=resid, scalar1=istd, scalar2=nbias,
                        op0=OP.mult, op1=OP.add,
                    )
                    if gbc is not None:
                        nc.vector.tensor_mul(z, z, gbc)
                    if bbc is not None:
                        nc.vector.tensor_add(z, z, bbc)
                    nc.vector.tensor_copy(out_tile, z)

            # ================= front: QKV + attention =================
            with (
                tc.tile_pool(name="p_ctxT", bufs=1) as p_ctxT,
                tc.tile_pool(name="p_ln1", bufs=3) as p_ln1,
                tc.tile_pool(name="p_w1", bufs=4) as p_w1,
            ):
                ctxT8 = p_ctxT.tile([128, KT, T], F8)
                wo8 = p_ctxT.tile([128, KT, C], F8)
                # xt / w1 pools live OUTSIDE the attention pools so their
                # DMAs have no address-reuse deps and land during attention
                xts = {}

                def xt_dma(m):
                    xt = p_ln1.tile([128, C], BF16, tag="xt",
                                    name=f"xt_{m}")
                    nc.sync.dma_start(
                        out=xt, in_=x_tc[m * 128:(m + 1) * 128, :]
                    )
                    xts[m] = xt

                w1cs = {}

                def w1_dma(jc):
                    w8c = p_w1.tile([128, 3, 2, 4, 128], F8, tag="w8c",
                                    name=f"w8c_{jc}")
                    nc.gpsimd.dma_start(
                        out=w8c, in_=w18_d[:, :, :, jc * 4:(jc + 1) * 4, :]
                    )
                    wtc = p_w1.tile([128, 2, 4, 128], BF16, tag="wtc",
                                    name=f"wtc_{jc}")
                    nc.gpsimd.dma_start(
                        out=wtc, in_=w1t_d[:, :, jc * 4:(jc + 1) * 4, :]
                    )
                    w1cs[jc] = (w8c, wtc)

                with (
                    tc.tile_pool(name="p_att", bufs=1) as p_att,
                    tc.tile_pool(name="p_ex", bufs=4) as p_ex,
                    tc.tile_pool(name="pp_big", bufs=5, space="PSUM") as pp_big,
                    tc.tile_pool(name="pp_ctx", bufs=2, space="PSUM") as pp_ctx,
                    tc.tile_pool(name="pp_tr", bufs=1, space="PSUM") as pp_tr,
                ):
                    qkT = p_att.tile([128, 16, T], BF16)
                    vsb = p_att.tile([128, MT, H * (HD + 1)], F8)
                    v4 = vsb.rearrange("p m (h e) -> p m h e", h=H)
                    ctxf = p_att.tile([128, MT, C], BF16)
                    xq8 = p_att.tile([128, KT, T], F8)
                    wqk8 = p_att.tile([128, 16, KT, 128], F8)
                    wv8 = p_att.tile([128, KT, C], F8)

                    nc.sync.dma_start(out=xq8[:, 0:2, :],
                                      in_=xq_d[:, 0:2, :])
                    nc.scalar.dma_start(out=xq8[:, 2:4, :],
                                      in_=xq_d[:, 2:4, :])
                    nc.gpsimd.dma_start(out=xq8[:, 4:6, :],
                                        in_=xq_d[:, 4:6, :])
                    nc.scalar.dma_start(out=xq8[:, 6:8, :],
                                      in_=xq_d[:, 6:8, :])
                    nc.sync.dma_start(out=wqk8[:, 0, :, :],
                                      in_=wqk_d[:, 0, :, :])
                    nc.sync.dma_start(out=wqk8[:, 8, :, :],
                                      in_=wqk_d[:, 8, :, :])
                    for lo, hi in ((1, 3), (9, 11), (3, 6), (11, 14),
                                   (6, 8), (14, 16)):
                        nc.sync.dma_start(out=wqk8[:, lo:hi, :, :],
                                          in_=wqk_d[:, lo:hi, :, :])
                    nc.scalar.dma_start(out=wv8, in_=wv_d[:, :, :])
                    nc.gpsimd.dma_start(out=wo8, in_=wo_d[:, :, :])
                    w1_dma(0)
                    w1_dma(1)
                    w1_dma(2)
                    w1_dma(3)
                    xt_dma(0)
                    xt_dma(1)
                    nc.vector.memset(v4[:, :, :, HD:HD + 1], 1.0)

                    def qkv_unit(m, n):
                        def u():
                            mm = pp_big.tile([128, 512], F32, tag="big")
                            for ks in range(4):
                                nc.tensor.matmul(
                                    mm,
                                    lhsT=wqk8[:, m, 2 * ks:2 * ks + 2, :],
                                    rhs=xq8[
                                        :, 2 * ks:2 * ks + 2,
                                        n * 512:(n + 1) * 512,
                                    ],
                                    start=(ks == 0), stop=(ks == 3),
                                    perf_mode=PM.DoubleRow,
                                )
                            nc.vector.tensor_copy(
                                qkT[:, m, n * 512:(n + 1) * 512], mm
                            )
                        return u

                    def vproj_unit(m, n):
                        def u():
                            mm = pp_big.tile([128, 512], F32, tag="big")
                            for ks in range(4):
                                nc.tensor.matmul(
                                    mm,
                                    lhsT=xq8[
                                        :, 2 * ks:2 * ks + 2,
                                        m * 128:(m + 1) * 128,
                                    ],
                                    rhs=wv8[
                                        :, 2 * ks:2 * ks + 2,
                                        n * 512:(n + 1) * 512,
                                    ],
                                    start=(ks == 0), stop=(ks == 3),
                                    perf_mode=PM.DoubleRow,
                                )
                            nc.vector.tensor_scalar_mul(
                                v4[:, m, n * 8:(n + 1) * 8, 0:HD],
                                mm.rearrange("p (h e) -> p h e", h=8),
                                1.0 / SV,
                            )
                        return u

                    exmap = {}

                    def sc_pair_units(a):
                        """Score+exp units for head pair (2a, 2a+1).  The even
                        head's kt/qt live on partitions 0-63, the odd head's on
                        64-127, so the two back-to-back matmuls land on disjoint
                        PE row groups (tile_position auto-derives) and execute
                        concurrently — 2x score throughput."""
                        he, ho = 2 * a, 2 * a + 1
                        exe = p_ex.tile([128, MT, T], F8, tag="ex",
                                        name=f"ex_{he}")
                        exo = p_ex.tile([128, MT, T], F8, tag="ex",
                                        name=f"ex_{ho}")
                        exmap[he], exmap[ho] = exe, exo
                        units = []
                        for j in range(MT):
                            qspan = T - j * 128
                            for lo in range(0, qspan, 512):
                                hi = min(lo + 512, qspan)

                                def u(j=j, lo=lo, hi=hi):
                                    qlo, qhi = j * 128 + lo, j * 128 + hi
                                    ste = pp_big.tile(
                                        [128, 512], F32, tag="big"
                                    )
                                    sto = pp_big.tile(
                                        [128, 512], F32, tag="big"
                                    )
                                    nc.tensor.matmul(
                                        ste[:, 0:hi - lo],
                                        lhsT=qkT[0:64, 8 + a,
                                                 j * 128:(j + 1) * 128],
                                        rhs=qkT[0:64, a, qlo:qhi],
                                        start=True, stop=True,
                                    )
                                    nc.tensor.matmul(
                                        sto[:, 0:hi - lo],
                                        lhsT=qkT[64:128, 8 + a,
                                                 j * 128:(j + 1) * 128],
                                        rhs=qkT[64:128, a, qlo:qhi],
                                        start=True, stop=True,
                                    )
                                    for exh, st in ((exe, ste), (exo, sto)):
                                        nc.scalar.activation(
                                            exh[:, j, qlo:qhi],
                                            st[:, 0:hi - lo],
                                            AF.Exp, bias=lnb, scale=ESC,
                                        )
                                        if lo == 0:
                                            nc.gpsimd.tensor_mul(
                                                exh[:, j,
                                                    j * 128:(j + 1) * 128],
                                                exh[:, j,
                                                    j * 128:(j + 1) * 128],
                                                mask01,
                                            )
                                units.append(u)
                        return units

                    def ctx_unit(h, i2):
                        exh = exmap[h]

                        def u():
                            # normal-mode fp8: 128-col weight loads keep the
                            # compiler's fast-weight-load (DoubleRow at free
                            # dim 65 is LDWEIGHTS-bound and congests the
                            # weight port for qkv/score loads)
                            cps = pp_ctx.tile([128, 4, HD + 1], F32, tag="cps")
                            for ii in range(4):
                                i = i2 * 4 + ii
                                for j in range(i + 1):
                                    nc.tensor.matmul(
                                        cps[:, ii, :],
                                        lhsT=exh[:, j, i * 128:(i + 1) * 128],
                                        rhs=v4[:, j, h, :],
                                        start=(j == 0), stop=(j == i),
                                    )
                            rden = smalls.tile([128, 4], F32, tag="rden")
                            nc.vector.reciprocal(
                                rden,
                                cps.rearrange("p i e -> p (i e)")[:, HD::HD + 1],
                            )
                            rb = bass.AP(
                                tensor=rden.tensor, offset=rden.offset,
                                ap=[rden.ap[0], [rden.ap[1][0], 4], [0, HD]],
                            )
                            nc.vector.tensor_mul(
                                ctxf[:, i2 * 4:(i2 + 1) * 4,
                                     h * HD:(h + 1) * HD],
                                cps[:, :, 0:HD], rb,
                            )
                        return u

                    def p4_unit(b, n):
                        def u():
                            tr = pp_tr.tile([128, 512], BF16, tag="tr")
                            for a in range(4):
                                nc.tensor.transpose(
                                    tr[:, a * 128:(a + 1) * 128],
                                    ctxf[:, n * 4 + a, b * 128:(b + 1) * 128],
                                    identb,
                                )
                            nc.vector.tensor_copy(
                                ctxT8[:, b, n * 512:(n + 1) * 512], tr
                            )
                        return u

                    # --- software-pipelined emission: paired score chunks
                    # (2x concurrent via row tiling) are spread uniformly
                    # between other PE work so the exp stream never starves;
                    # qkT pairs are produced one slot ahead. ---
                    for u in (qkv_unit(0, 0), qkv_unit(0, 1),
                              qkv_unit(8, 0), qkv_unit(8, 1)):
                        u()
                    for a in range(8):
                        # clean big matmuls (qkv/vproj) interleave with the
                        # score pairs; the many small ctx/p4 weight loads go
                        # in a suffix block so they stop colliding with the
                        # score kt loads on the weight port
                        others = []
                        if a + 1 < 8:
                            others += [qkv_unit(a + 1, 0), qkv_unit(a + 1, 1),
                                       qkv_unit(9 + a, 0), qkv_unit(9 + a, 1)]
                        if a == 0:
                            others += [vproj_unit(m, n)
                                       for m in range(4) for n in range(2)]
                        elif a == 1:
                            others += [vproj_unit(m, n)
                                       for m in range(4, 8) for n in range(2)]
                        suffix = []
                        if a >= 1:
                            hp = 2 * (a - 1)
                            suffix += [ctx_unit(hp, 0), ctx_unit(hp, 1),
                                       ctx_unit(hp + 1, 0), ctx_unit(hp + 1, 1)]
                        if a >= 2:
                            suffix += [p4_unit(a - 2, 0), p4_unit(a - 2, 1)]
                        sts = sc_pair_units(a)
                        ns = len(sts)
                        no = len(others)
                        for idx, su in enumerate(sts):
                            su()
                            for u in others[
                                (idx * no) // ns:((idx + 1) * no) // ns
                            ]:
                                u()
                        for u in suffix:
                            u()
                    for u in (ctx_unit(H - 2, 0), ctx_unit(H - 2, 1),
                              ctx_unit(H - 1, 0), ctx_unit(H - 1, 1),
                              p4_unit(6, 0), p4_unit(6, 1),
                              p4_unit(7, 0), p4_unit(7, 1)):
                        u()

                # ---- out-proj + residual + LN1 + h^T + FFN1 overlap ----
                # Residual rides the PE: the out-proj PSUM group starts with
                # identb.T @ (512*x)  (bf16), then accumulates the fp8 DR
                # out-proj matmuls.  LN is scale-invariant, so the 512x
                # scale needs no unscale; bn_stats reads the PSUM directly.
                with (
                    tc.tile_pool(name="p_ff1", bufs=1) as p_ff1,
                    tc.tile_pool(name="p_w2r", bufs=1) as p_w2r,
                ):
                  ff1 = p_ff1.tile([128, JT, T], BF16)
                  w2sb = p_w2r.tile([128, JT, C], BF16)

                  def w2_dma(jc):
                      nc.gpsimd.dma_start(
                          out=w2sb[:, jc * 4:(jc + 1) * 4, :],
                          in_=w2_d[:, jc * 4:(jc + 1) * 4, :],
                      )

                  with (
                    tc.tile_pool(name="pp_ao", bufs=2, space="PSUM") as pp_ao,
                    tc.tile_pool(name="pp_f1", bufs=3, space="PSUM") as pp_f1,
                    tc.tile_pool(name="pp_tr7", bufs=1, space="PSUM") as pp_tr7,
                  ):
                    def op_ln1(m):
                        ao = pp_ao.tile([128, C], F32, tag="ao")
                        for n in range(2):
                            nc.tensor.matmul(
                                ao[:, n * 512:(n + 1) * 512],
                                lhsT=identb,
                                rhs=xts[m][:, n * 512:(n + 1) * 512],
                                start=True, stop=False,
                            )
                            for ks in range(4):
                                nc.tensor.matmul(
                                    ao[:, n * 512:(n + 1) * 512],
                                    lhsT=ctxT8[
                                        :, 2 * ks:2 * ks + 2,
                                        m * 128:(m + 1) * 128,
                                    ],
                                    rhs=wo8[
                                        :, 2 * ks:2 * ks + 2,
                                        n * 512:(n + 1) * 512,
                                    ],
                                    start=False, stop=(ks == 3),
                                    perf_mode=PM.DoubleRow,
                                )
                        layernorm(ao, hbf[:, m, :], g1bc, b1lnbc, p_ln1)

                    def ht_half(n):
                        for b in range(KT):
                            tr = pp_tr7.tile([128, 512], BF16, tag="tr7")
                            for a in range(4):
                                nc.tensor.transpose(
                                    tr[:, a * 128:(a + 1) * 128],
                                    hbf[:, n * 4 + a, b * 128:(b + 1) * 128],
                                    identb,
                                )
                            if b < 6:
                                nc.vector.tensor_scalar_mul(
                                    hT8[:, b, n * 512:(n + 1) * 512], tr, 8.0
                                )
                            else:
                                nc.vector.tensor_scalar_mul(
                                    hTb[:, b - 6, n * 512:(n + 1) * 512],
                                    tr, 16.0,
                                )

                    def f1_half(j, n, dve_relu=False):
                        ps = pp_f1.tile([128, 512], F32, tag="f1h")
                        w8c, wtc = w1cs[j // 4]
                        jj = j % 4
                        for p in range(3):
                            nc.tensor.matmul(
                                ps,
                                lhsT=w8c[:, p, :, jj, :],
                                rhs=hT8[:, 2 * p:2 * p + 2,
                                        n * 512:(n + 1) * 512],
                                start=(p == 0), stop=False,
                                perf_mode=PM.DoubleRow,
                            )
                        for k in range(2):
                            nc.tensor.matmul(
                                ps,
                                lhsT=wtc[:, k, jj, :],
                                rhs=hTb[:, k, n * 512:(n + 1) * 512],
                                start=False, stop=(k == 1),
                            )
                        if dve_relu and not has_b1:
                            # LN1 window: ACT does the LN chain and DVE the
                            # stats there; run these fillers' ReLU on the
                            # idle GpSimd so the f1 PSUM slots recycle
                            # without delaying either chain engine
                            nc.vector.tensor_scalar(
                                out=ff1[:, j, n * 512:(n + 1) * 512],
                                in0=ps, scalar1=1.0 / 256.0, scalar2=0.0,
                                op0=OP.mult, op1=OP.max,
                            )
                            return
                        bias = b1sb[:, j:j + 1] if has_b1 else 0.0
                        nc.scalar.activation(
                            ff1[:, j, n * 512:(n + 1) * 512], ps,
                            AF.Relu, bias=bias, scale=1.0 / 256.0,
                        )

                    for m in range(MT):
                        if m + 2 < MT:
                            xt_dma(m + 2)
                        op_ln1(m)
                        if m == 3:
                            ht_half(0)
                        elif m >= 4:
                            f1_half(m - 4, 0, dve_relu=True)
                    for j in range(4, 16):
                        f1_half(j, 0, dve_relu=True)
                    ht_half(1)
                    w2_dma(0)
                    for jc in range(4):
                        for jj in range(4):
                            f1_half(jc * 4 + jj, 1)
                        w1_dma(4 + jc)
                        w2_dma(1 + jc)
                    for jc in range(4, 8):
                        if jc < 7:
                            w2_dma(jc + 1)
                        for jj in range(4):
                            j = jc * 4 + jj
                            f1_half(j, 0)
                            f1_half(j, 1)

                  # ---- FFN2 + residual + LN2 + out^T ----
                  with (
                    tc.tile_pool(name="p_ln2", bufs=2) as p_ln2,
                    tc.tile_pool(name="p_z2", bufs=2) as p_z2,
                    tc.tile_pool(name="p_out", bufs=4) as p_out,
                    tc.tile_pool(name="pp_f2", bufs=1, space="PSUM") as pp_f2,
                    tc.tile_pool(name="pp_t11", bufs=2, space="PSUM") as pp_t11,
                  ):
                    def ffn2_m(m):
                        ps = pp_f2.tile([128, C], F32, tag=f"f2_{m % 2}",
                                        name=f"f2ps_{m}")
                        for n in range(2):
                            nc.tensor.matmul(
                                ps[:, n * 512:(n + 1) * 512],
                                lhsT=identb,
                                rhs=hbf[:, m, n * 512:(n + 1) * 512],
                                start=True, stop=False,
                            )
                            for j in range(JT):
                                nc.tensor.matmul(
                                    ps[:, n * 512:(n + 1) * 512],
                                    lhsT=ff1[:, j, m * 128:(m + 1) * 128],
                                    rhs=w2sb[:, j, n * 512:(n + 1) * 512],
                                    start=False, stop=(j == JT - 1),
                                )
                        return ps

                    def ln2_m(m, ps):
                        if has_b2:
                            resid2 = p_ln2.tile([128, C], F32, tag="resid2")
                            nc.vector.tensor_add(resid2, ps, b2bc)
                            src = resid2
                        else:
                            src = ps
                        zt = p_z2.tile([128, C], BF16, tag="z2",
                                       name=f"z2_{m}")
                        layernorm(src, zt, g2bc, b2lnbc, p_ln2)
                        return zt

                    def outT(m, zt):
                        # bf16 transposes (final LN2 output; ~0.1% rounding)
                        for b in range(KT):
                            tr = pp_t11.tile([128, 128], BF16, tag="t11")
                            nc.tensor.transpose(
                                tr, zt[:, b * 128:(b + 1) * 128], identb,
                            )
                            osb = p_out.tile([128, 128], F32, tag="osb")
                            nc.vector.tensor_copy(osb, tr)
                            eng = nc.sync if b % 2 == 0 else nc.scalar
                            eng.dma_start(
                                out=out_d[
                                    b * 128:(b + 1) * 128,
                                    m * 128:(m + 1) * 128,
                                ],
                                in_=osb,
                            )

                    for m in range(MT):
                        ps = ffn2_m(m)
                        outT(m, ln2_m(m, ps))
    _split_multiwait(nc)
    return nc


_prog_cache = {}


def prepare(
    x, in_proj_w, out_proj_w, ln1_g, ln1_b, ln2_g, ln2_b, w1, b1, w2, b2
):
    """Returns (nc, in_maps): the built program plus per-core input maps."""
    x = np.asarray(x, dtype=np.float32)
    in_proj_w = np.asarray(in_proj_w, dtype=np.float32)
    out_proj_w = np.asarray(out_proj_w, dtype=np.float32)
    ln1_g = np.asarray(ln1_g, dtype=np.float32)
    ln1_b = np.asarray(ln1_b, dtype=np.float32)
    ln2_g = np.asarray(ln2_g, dtype=np.float32)
    ln2_b = np.asarray(ln2_b, dtype=np.float32)
    w1 = np.asarray(w1, dtype=np.float32)
    b1 = np.asarray(b1, dtype=np.float32)
    w2 = np.asarray(w2, dtype=np.float32)
    b2 = np.asarray(b2, dtype=np.float32)

    flags = (
        not np.all(ln1_g == 1.0),
        not np.all(ln1_b == 0.0),
        not np.all(ln2_g == 1.0),
        not np.all(ln2_b == 0.0),
        not np.all(b1 == 0.0),
        not np.all(b2 == 0.0),
    )
    if flags not in _prog_cache:
        _prog_cache[flags] = _build(flags)
    nc = _prog_cache[flags]

    peT, ident, mask01 = _host_constants()

    winT = np.ascontiguousarray(in_proj_w.T)             # [C, 3C] f32
    wqk = np.ascontiguousarray(np.transpose(
        (winT[:, :2 * C] * SW).reshape(KT, 128, 16, 128), (1, 2, 0, 3)
    )).astype(E4)                                        # [128, 16, KT, 128]
    wv = np.ascontiguousarray(np.transpose(
        (winT[:, 2 * C:] * SW).reshape(KT, 128, C), (1, 0, 2)
    )).astype(E4)                                        # [128, KT, C]
    wo = np.ascontiguousarray(np.transpose(
        (out_proj_w.T * SWO).reshape(KT, 128, C), (1, 0, 2)
    )).astype(E4)                                        # [128, KT, C]
    w1r = np.transpose(
        w1.T.reshape(KT, 128, JT, 128), (1, 0, 2, 3)
    )                                                    # [128, KT, JT, 128]
    w18 = np.ascontiguousarray(
        (w1r[:, 0:6] * 32.0).reshape(128, 3, 2, JT, 128)
    ).astype(E4)                                         # [128, 3, 2, JT, 128]
    w1t = np.ascontiguousarray(w1r[:, 6:8] * 16.0).astype(BF)
    w2b = np.ascontiguousarray(np.transpose(
        w2.T.reshape(JT, 128, C), (1, 0, 2)
    )).astype(BF)                                        # [128, JT, C]

    shared = {
        "wqk8": wqk,
        "wv8": wv,
        "wo8": wo,
        "w18": w18,
        "w1t": w1t,
        "w2b": w2b,
        "identf": ident.astype(np.float32),
        "identb": ident.astype(BF),
        "mask01": mask01,
    }
    if flags[0]:
        shared["g1"] = ln1_g
    if flags[1]:
        shared["b1ln"] = ln1_b
    if flags[2]:
        shared["g2"] = ln2_g
    if flags[3]:
        shared["b2ln"] = ln2_b
    if flags[4]:
        shared["b1t"] = np.ascontiguousarray(b1.reshape(JT, 128).T)
    if flags[5]:
        shared["b2"] = b2

    in_maps = []
    for i in range(N):
        m = dict(shared)
        xq = (x[i] + peT) * SX                            # [C, T]
        m["xq8"] = np.ascontiguousarray(
            np.transpose(xq.reshape(KT, 128, T), (1, 0, 2))
        ).astype(E4)                                      # [128, KT, T]
        # residual ships pre-scaled by SV*SWO (the attention-psum scale;
        # LN is scale-invariant) so it can ride the out-proj matmul group
        m["x_tc"] = np.ascontiguousarray(
            x[i].T * (SV * SWO)
        ).astype(BF)                                      # [T, C] bf16
        in_maps.append(m)
    return nc, in_maps


def kernel(**inputs):
    nc, in_maps = prepare(**inputs)
    res = run_bass_kernel_spmd(nc, in_maps, core_ids=list(range(N)))
    out = np.stack([res.results[i]["out"] for i in range(N)], axis=0)
    return out.astype(np.float32)



# revision 57
# speedup vs baseline: 1.1245x; 1.1245x over previous
"""Trainium2 Bass kernel for nn_MhaSelfAttenLayer (dense transformer layer).

Data-parallel over batch: each of the 8 NeuronCores runs the full layer on
one batch element. No collectives.

Precision plan (validated numerically; sim matches HW to ~1e-4):
- Attention path (QKV proj, V, out proj, attn*V) in fp8 e4m3 with DoubleRow
  matmuls; scores in bf16 (64-wide contraction gets no DoubleRow benefit);
  fp32 PSUM accumulation everywhere.
- FFN1 contracts 6 of 8 c-tiles in fp8 DoubleRow (x32/x8 scales) and the
  last 2 in bf16 (x16/x16, so both partials carry the same 256x PSUM
  scale, removed in the ReLU activation scale).  7/8 or 8/8 fp8 breaches
  the 2e-2 budget; 6/8 lands at 1.89e-2.  FFN2 stays bf16.
- Residual adds ride the PE: each out-proj / FFN2 PSUM accumulation group
  starts with identity.T @ residual (bf16), and the LayerNorms read stats
  straight from PSUM.  LN is scale-invariant, so the fp8 512x scale on the
  attention PSUM needs no unscale; x ships pre-scaled as bf16.
- Scale folding: qkT holds 256*q ; the 256^2 factor is removed inside the
  exp() activation scale (exact power of two), v/ctx/out-proj scales fold
  into existing copies, so fp8 costs no extra instructions.
"""

import math

import numpy as np
import ml_dtypes

import concourse.bass as bass
import concourse.tile as tile
from concourse import mybir
from concourse.bass_utils import run_bass_kernel_spmd
from concourse.vector_clock import ScopedClock, VectorClock

F32 = mybir.dt.float32
BF16 = mybir.dt.bfloat16
F8 = mybir.dt.float8e4
BF = ml_dtypes.bfloat16
E4 = ml_dtypes.float8_e4m3

N, T, C, H, HD, HID = 8, 1024, 1024, 16, 64, 4096
KT = C // 128           # 8 c-tiles
MT = T // 128           # 8 t-tiles
JT = HID // 128         # 32 hid-tiles
EPS = 1e-5
AF = mybir.ActivationFunctionType
OP = mybir.AluOpType
PM = mybir.MatmulPerfMode

SX = 8.0                # xq8 = SX*(x+pe)
SW = 32.0               # wqk8 = SW*w  -> qkT = 256*q
ESC = 1.0 / (256.0 * 256.0 * 8.0)   # exp arg unscale (2^-19), incl 1/sqrt(hd)
SE = 2.0                # ex = SE*exp(score)
SV = 16.0               # v8 = SV*v ; ctxT8 = SV*ctx
SWO = 32.0              # wo8 = SWO*wo
AOS = 1.0 / (SV * SWO)  # attn-out unscale

_patched = False


def _patch_drain():
    """This walrus build accepts at most 1 sem wait per instruction (2 for
    EventSemaphore). Tile's final drain packs every outstanding proc wait
    onto a single drain -> codegen error. Emit one drain per proc instead."""
    global _patched
    if _patched:
        return
    _patched = True

    def _split_drain_and_barrier(self, tick_clock, wait_clock):
        gclock = tick_clock.global_clock
        n = len(gclock)
        for proc in range(n):
            t = gclock[proc]
            if t <= 0:
                continue
            vc = VectorClock([0] * n)
            vc.require_at_least(proc, t)
            d = self.nc.sync.drain()
            wait_clock.add_sem_waits(d.ins, ScopedClock({None: vc}))
        self.nc.all_engine_barrier()
        popped = self.nc._tile_sem_poison_stack.pop()
        assert popped is self._sem_poison
        self.nc.clear_and_free_semaphores(list(self.sems.allocated().values()))
        self.nc.all_engine_barrier()

    tile.TileContext._drain_and_barrier = _split_drain_and_barrier


def _split_multiwait(nc):
    """This walrus build accepts at most one sem wait per instruction. Hoist
    excess waits onto freshly created same-engine nops placed immediately
    before the over-limit instruction (engine streams run in order, so the
    nop blocking first preserves the dependency)."""
    import bass_rust

    engmap = {
        mybir.EngineType.PE: nc.tensor,
        mybir.EngineType.DVE: nc.vector,
        mybir.EngineType.Activation: nc.scalar,
        mybir.EngineType.SP: nc.sync,
        mybir.EngineType.Pool: nc.gpsimd,
    }
    blocks = list(nc.main_func.blocks)
    records = []
    for bi, bb in enumerate(blocks):
        for ins in bb.instructions:
            si = ins.sync_info
            if si is None or not si.on_wait:
                continue
            waits = list(si.on_wait)
            limit = 2 if type(ins).__name__ == "InstEventSemaphore" else 1
            if len(waits) > limit:
                records.append((ins.name, ins, waits[:-limit]))
                si.on_wait = waits[-limit:]
    if not records:
        return
    carriers = {}
    nop_names = set()
    for name, ins, excess in records:
        lst = []
        for w in excess:
            nb = engmap[ins.engine].nop()
            nb.ins.sync_info = bass_rust.SyncInfo(on_wait=[w], on_update=[])
            nop_names.add(nb.ins.name)
            lst.append(nb.ins)
        carriers[name] = lst
    for bb in blocks:
        il = list(bb.instructions)
        out = []
        changed = False
        for ins in il:
            if ins.name in nop_names:
                changed = True
                continue
            if ins.name in carriers:
                out.extend(carriers[ins.name])
                changed = True
            out.append(ins)
        if changed:
            bb.instructions = out


def _host_constants():
    pos = np.arange(T, dtype=np.float32)[:, None]
    div = np.exp(
        np.arange(0, C, 2, dtype=np.float32) * (-math.log(10000.0) / C)
    )
    ang = pos * div
    pe = np.stack([np.sin(ang), np.cos(ang)], axis=-1).reshape(T, C)
    peT = np.ascontiguousarray(pe.T)                    # [C, T]

    ident = np.eye(128, dtype=np.float32)
    kk = np.arange(128)
    # mask01[k, q] = 0 where q < k (future key within diagonal block)
    mask01 = np.where(kk[None, :] < kk[:, None], 0.0, 1.0).astype(E4)
    return peT, ident, mask01


def _build(flags):
    """flags = (g1, b1ln, g2, b2ln, b1, b2) booleans for non-trivial params."""
    has_g1, has_b1ln, has_g2, has_b2ln, has_b1, has_b2 = flags
    _patch_drain()
    nc = bass.Bass(trn_type="TRN2")

    # ---- DRAM I/O ----
    xq_d = nc.dram_tensor("xq8", [128, KT, T], F8, kind="ExternalInput")
    x_tc = nc.dram_tensor("x_tc", [T, C], BF16, kind="ExternalInput")
    wqk_d = nc.dram_tensor("wqk8", [128, 16, KT, 128], F8, kind="ExternalInput")
    wv_d = nc.dram_tensor("wv8", [128, KT, C], F8, kind="ExternalInput")
    wo_d = nc.dram_tensor("wo8", [128, KT, C], F8, kind="ExternalInput")
    w18_d = nc.dram_tensor("w18", [128, 3, 2, JT, 128], F8,
                           kind="ExternalInput")
    w1t_d = nc.dram_tensor("w1t", [128, 2, JT, 128], BF16,
                           kind="ExternalInput")
    w2_d = nc.dram_tensor("w2b", [128, JT, C], BF16, kind="ExternalInput")
    idb_d = nc.dram_tensor("identb", [128, 128], BF16, kind="ExternalInput")
    mk_d = nc.dram_tensor("mask01", [128, 128], F8, kind="ExternalInput")
    if has_g1:
        g1_d = nc.dram_tensor("g1", [C], F32, kind="ExternalInput")
    if has_b1ln:
        b1ln_d = nc.dram_tensor("b1ln", [C], F32, kind="ExternalInput")
    if has_g2:
        g2_d = nc.dram_tensor("g2", [C], F32, kind="ExternalInput")
    if has_b2ln:
        b2ln_d = nc.dram_tensor("b2ln", [C], F32, kind="ExternalInput")
    if has_b1:
        b1_d = nc.dram_tensor("b1t", [128, JT], F32, kind="ExternalInput")
    if has_b2:
        b2_d = nc.dram_tensor("b2", [C], F32, kind="ExternalInput")
    out_d = nc.dram_tensor("out", [C, T], F32, kind="ExternalOutput")

    def bcast_ap(dram_1d, n):
        return bass.AP(tensor=dram_1d.tensor, offset=0, ap=[[0, 128], [1, n]])

    with tile.TileContext(nc) as tc:
        with (
            tc.tile_pool(name="consts", bufs=1) as consts,
            tc.tile_pool(name="smalls", bufs=12) as smalls,
            tc.tile_pool(name="p_hbf", bufs=1) as p_hbf,
            tc.tile_pool(name="p_hT", bufs=1) as p_hT,
        ):
            # ---- constants (ACT-ring DMAs; SP ring is kept clear for the
            # latency-critical xq8/wqk loads) ----
            zbias = consts.tile([128, 1], F32)
            nc.vector.memset(zbias, 0.0)
            nc.const_aps.aps[(F32, 0.0)] = zbias
            epsb = consts.tile([128, 1], F32)
            nc.vector.memset(epsb, EPS)
            lnb = consts.tile([128, 1], F32)
            nc.vector.memset(lnb, float(math.log(SE)))
            warm = consts.tile([128, 64], BF16)
            nc.vector.memset(warm, 0.25)
            tblw = smalls.tile([128, 1], F32, tag="tblw")
            nc.scalar.activation(tblw, epsb, AF.Exp, bias=0.0, scale=1.0)
            nc.scalar.activation(tblw, epsb, AF.Sqrt, bias=0.0, scale=1.0)
            mask01 = consts.tile([128, 128], F8)
            nc.gpsimd.dma_start(out=mask01, in_=mk_d[:, :])
            identb = consts.tile([128, 128], BF16)
            nc.gpsimd.dma_start(out=identb, in_=idb_d[:, :])
            g1bc = b1lnbc = g2bc = b2lnbc = b1sb = b2bc = None
            if has_g1:
                g1bc = consts.tile([128, C], F32)
                nc.scalar.dma_start(out=g1bc, in_=bcast_ap(g1_d, C))
            if has_b1ln:
                b1lnbc = consts.tile([128, C], F32)
                nc.scalar.dma_start(out=b1lnbc, in_=bcast_ap(b1ln_d, C))
            if has_g2:
                g2bc = consts.tile([128, C], F32)
                nc.scalar.dma_start(out=g2bc, in_=bcast_ap(g2_d, C))
            if has_b2ln:
                b2lnbc = consts.tile([128, C], F32)
                nc.scalar.dma_start(out=b2lnbc, in_=bcast_ap(b2ln_d, C))
            if has_b1:
                b1sb = consts.tile([128, JT], F32)
                nc.scalar.dma_start(out=b1sb, in_=b1_d[:, :])
            if has_b2:
                b2bc = consts.tile([128, C], F32)
                nc.scalar.dma_start(out=b2bc, in_=bcast_ap(b2_d, C))

            # ---- PE warm-up: ~6us of tiny matmuls during the input-DMA
            # wait so the HAM clock-gate reaches K=8/8 before real work ----
            with tc.tile_pool(name="pp_warm", bufs=1, space="PSUM") as pp_w:
                wps = pp_w.tile([64, 64], F32)
                for _ in range(84):
                    nc.tensor.matmul(wps, lhsT=warm[:, 0:64],
                                     rhs=warm[:, 0:64], start=True, stop=True)

            hbf = p_hbf.tile([128, MT, C], BF16)
            hT8 = p_hT.tile([128, 6, T], F8)
            hTb = p_hT.tile([128, 2, T], BF16)

            def layernorm(resid, out_tile, gbc, bbc, zpool):
                stats = smalls.tile([128, 2, 6], F32, tag="stats")
                nc.vector.bn_stats(out=stats[:, 0, :], in_=resid[:, 0:512])
                nc.vector.bn_stats(out=stats[:, 1, :], in_=resid[:, 512:1024])
                mv = smalls.tile([128, 2], F32, tag="mv")
                nc.vector.bn_aggr(out=mv, in_=stats)
                std = smalls.tile([128, 1], F32, tag="std")
                nc.scalar.activation(std, mv[:, 1:2], AF.Sqrt, bias=epsb)
                istd = smalls.tile([128, 1], F32, tag="istd")
                nc.vector.reciprocal(istd, std)
                nbias = smalls.tile([128, 1], F32, tag="nbias")
                nc.vector.tensor_scalar(
                    out=nbias, in0=mv[:, 0:1], scalar1=istd, scalar2=-1.0,
                    op0=OP.mult, op1=OP.mult,
                )
                if gbc is None and bbc is None:
                    # two halves: downstream per-128-col transposes unblock
                    # after half 0 (subtile deps), hiding the chain latency
                    nc.scalar.activation(
                        out_tile[:, 0:512], resid[:, 0:512],
                        AF.Identity, bias=nbias, scale=istd,
                    )
                    nc.scalar.activation(
                        out_tile[:, 512:1024], resid[:, 512:1024],
                        AF.Identity, bias=nbias, scale=istd,
                    )
                else:
                    z = zpool.tile([128, C], F32, tag="zln")
                    nc.vector.tensor_scalar(
                        out=z, in0=resid, scalar1=istd, scalar2=nbias,
                        op0=OP.mult, op1=OP.add,
                    )
                    if gbc is not None:
                        nc.vector.tensor_mul(z, z, gbc)
                    if bbc is not None:
                        nc.vector.tensor_add(z, z, bbc)
                    nc.vector.tensor_copy(out_tile, z)

            # ================= front: QKV + attention =================
            with (
                tc.tile_pool(name="p_ctxT", bufs=1) as p_ctxT,
                tc.tile_pool(name="p_ln1", bufs=3) as p_ln1,
                tc.tile_pool(name="p_w1", bufs=4) as p_w1,
            ):
                ctxT8 = p_ctxT.tile([128, KT, T], F8)
                wo8 = p_ctxT.tile([128, KT, C], F8)
                # xt / w1 pools live OUTSIDE the attention pools so their
                # DMAs have no address-reuse deps and land during attention
                xts = {}

                def xt_dma(m):
                    xt = p_ln1.tile([128, C], BF16, tag="xt",
                                    name=f"xt_{m}")
                    nc.sync.dma_start(
                        out=xt, in_=x_tc[m * 128:(m + 1) * 128, :]
                    )
                    xts[m] = xt

                w1cs = {}

                def w1_dma(jc):
                    w8c = p_w1.tile([128, 3, 2, 4, 128], F8, tag="w8c",
                                    name=f"w8c_{jc}")
                    nc.gpsimd.dma_start(
                        out=w8c, in_=w18_d[:, :, :, jc * 4:(jc + 1) * 4, :]
                    )
                    wtc = p_w1.tile([128, 2, 4, 128], BF16, tag="wtc",
                                    name=f"wtc_{jc}")
                    nc.gpsimd.dma_start(
                        out=wtc, in_=w1t_d[:, :, jc * 4:(jc + 1) * 4, :]
                    )
                    w1cs[jc] = (w8c, wtc)

                with (
                    tc.tile_pool(name="p_att", bufs=1) as p_att,
                    tc.tile_pool(name="p_ex", bufs=4) as p_ex,
                    tc.tile_pool(name="pp_big", bufs=5, space="PSUM") as pp_big,
                    tc.tile_pool(name="pp_ctx", bufs=2, space="PSUM") as pp_ctx,
                    tc.tile_pool(name="pp_tr", bufs=1, space="PSUM") as pp_tr,
                ):
                    qkT = p_att.tile([128, 16, T], BF16)
                    vsb = p_att.tile([128, MT, H * (HD + 1)], F8)
                    v4 = vsb.rearrange("p m (h e) -> p m h e", h=H)
                    ctxf = p_att.tile([128, MT, C], BF16)
                    xq8 = p_att.tile([128, KT, T], F8)
                    wqk8 = p_att.tile([128, 16, KT, 128], F8)
                    wv8 = p_att.tile([128, KT, C], F8)

                    nc.sync.dma_start(out=xq8[:, 0:2, :],
                                      in_=xq_d[:, 0:2, :])
                    nc.scalar.dma_start(out=xq8[:, 2:4, :],
                                      in_=xq_d[:, 2:4, :])
                    nc.gpsimd.dma_start(out=xq8[:, 4:6, :],
                                        in_=xq_d[:, 4:6, :])
                    nc.scalar.dma_start(out=xq8[:, 6:8, :],
                                      in_=xq_d[:, 6:8, :])
                    nc.sync.dma_start(out=wqk8[:, 0, :, :],
                                      in_=wqk_d[:, 0, :, :])
                    nc.sync.dma_start(out=wqk8[:, 8, :, :],
                                      in_=wqk_d[:, 8, :, :])
                    for lo, hi in ((1, 3), (9, 11), (3, 6), (11, 14),
                                   (6, 8), (14, 16)):
                        nc.sync.dma_start(out=wqk8[:, lo:hi, :, :],
                                          in_=wqk_d[:, lo:hi, :, :])
                    nc.scalar.dma_start(out=wv8, in_=wv_d[:, :, :])
                    nc.gpsimd.dma_start(out=wo8, in_=wo_d[:, :, :])
                    w1_dma(0)
                    w1_dma(1)
                    w1_dma(2)
                    w1_dma(3)
                    xt_dma(0)
                    xt_dma(1)
                    nc.vector.memset(v4[:, :, :, HD:HD + 1], 1.0)

                    def qkv_unit(m, n):
                        def u():
                            mm = pp_big.tile([128, 512], F32, tag="big")
                            for ks in range(4):
                                nc.tensor.matmul(
                                    mm,
                                    lhsT=wqk8[:, m, 2 * ks:2 * ks + 2, :],
                                    rhs=xq8[
                                        :, 2 * ks:2 * ks + 2,
                                        n * 512:(n + 1) * 512,
                                    ],
                                    start=(ks == 0), stop=(ks == 3),
                                    perf_mode=PM.DoubleRow,
                                )
                            nc.vector.tensor_copy(
                                qkT[:, m, n * 512:(n + 1) * 512], mm
                            )
                        return u

                    def vproj_unit(m, n):
                        def u():
                            mm = pp_big.tile([128, 512], F32, tag="big")
                            for ks in range(4):
                                nc.tensor.matmul(
                                    mm,
                                    lhsT=xq8[
                                        :, 2 * ks:2 * ks + 2,
                                        m * 128:(m + 1) * 128,
                                    ],
                                    rhs=wv8[
                                        :, 2 * ks:2 * ks + 2,
                                        n * 512:(n + 1) * 512,
                                    ],
                                    start=(ks == 0), stop=(ks == 3),
                                    perf_mode=PM.DoubleRow,
                                )
                            nc.vector.tensor_scalar_mul(
                                v4[:, m, n * 8:(n + 1) * 8, 0:HD],
                                mm.rearrange("p (h e) -> p h e", h=8),
                                1.0 / SV,
                            )
                        return u

                    exmap = {}

                    def sc_pair_units(a):
                        """Score+exp units for head pair (2a, 2a+1).  The even
                        head's kt/qt live on partitions 0-63, the odd head's on
                        64-127, so the two back-to-back matmuls land on disjoint
                        PE row groups (tile_position auto-derives) and execute
                        concurrently — 2x score throughput."""
                        he, ho = 2 * a, 2 * a + 1
                        exe = p_ex.tile([128, MT, T], F8, tag="ex",
                                        name=f"ex_{he}")
                        exo = p_ex.tile([128, MT, T], F8, tag="ex",
                                        name=f"ex_{ho}")
                        exmap[he], exmap[ho] = exe, exo
                        units = []
                        for j in range(MT):
                            qspan = T - j * 128
                            for lo in range(0, qspan, 512):
                                hi = min(lo + 512, qspan)

                                def u(j=j, lo=lo, hi=hi):
                                    qlo, qhi = j * 128 + lo, j * 128 + hi
                                    ste = pp_big.tile(
                                        [128, 512], F32, tag="big"
                                    )
                                    sto = pp_big.tile(
                                        [128, 512], F32, tag="big"
                                    )
                                    nc.tensor.matmul(
                                        ste[:, 0:hi - lo],
                                        lhsT=qkT[0:64, 8 + a,
                                                 j * 128:(j + 1) * 128],
                                        rhs=qkT[0:64, a, qlo:qhi],
                                        start=True, stop=True,
                                    )
                                    nc.tensor.matmul(
                                        sto[:, 0:hi - lo],
                                        lhsT=qkT[64:128, 8 + a,
                                                 j * 128:(j + 1) * 128],
                                        rhs=qkT[64:128, a, qlo:qhi],
                                        start=True, stop=True,
                                    )
                                    for exh, st in ((exe, ste), (exo, sto)):
                                        nc.scalar.activation(
                                            exh[:, j, qlo:qhi],
                                            st[:, 0:hi - lo],
                                            AF.Exp, bias=lnb, scale=ESC,
                                        )
                                        if lo == 0:
                                            nc.gpsimd.tensor_mul(
                                                exh[:, j,
                                                    j * 128:(j + 1) * 128],
                                                exh[:, j,
                                                    j * 128:(j + 1) * 128],
                                                mask01,
                                            )
                                units.append(u)
                        return units

                    def ctx_unit(h, i2):
                        exh = exmap[h]

                        def u():
                            # normal-mode fp8: 128-col weight loads keep the
                            # compiler's fast-weight-load (DoubleRow at free
                            # dim 65 is LDWEIGHTS-bound and congests the
                            # weight port for qkv/score loads)
                            cps = pp_ctx.tile([128, 4, HD + 1], F32, tag="cps")
                            for ii in range(4):
                                i = i2 * 4 + ii
                                for j in range(i + 1):
                                    nc.tensor.matmul(
                                        cps[:, ii, :],
                                        lhsT=exh[:, j, i * 128:(i + 1) * 128],
                                        rhs=v4[:, j, h, :],
                                        start=(j == 0), stop=(j == i),
                                    )
                            rden = smalls.tile([128, 4], F32, tag="rden")
                            nc.vector.reciprocal(
                                rden,
                                cps.rearrange("p i e -> p (i e)")[:, HD::HD + 1],
                            )
                            rb = bass.AP(
                                tensor=rden.tensor, offset=rden.offset,
                                ap=[rden.ap[0], [rden.ap[1][0], 4], [0, HD]],
                            )
                            nc.vector.tensor_mul(
                                ctxf[:, i2 * 4:(i2 + 1) * 4,
                                     h * HD:(h + 1) * HD],
                                cps[:, :, 0:HD], rb,
                            )
                        return u

                    def p4_unit(b, n):
                        def u():
                            tr = pp_tr.tile([128, 512], BF16, tag="tr")
                            for a in range(4):
                                nc.tensor.transpose(
                                    tr[:, a * 128:(a + 1) * 128],
                                    ctxf[:, n * 4 + a, b * 128:(b + 1) * 128],
                                    identb,
                                )
                            nc.vector.tensor_copy(
                                ctxT8[:, b, n * 512:(n + 1) * 512], tr
                            )
                        return u

                    # --- software-pipelined emission: paired score chunks
                    # (2x concurrent via row tiling) are spread uniformly
                    # between other PE work so the exp stream never starves;
                    # qkT pairs are produced one slot ahead. ---
                    for u in (qkv_unit(0, 0), qkv_unit(0, 1),
                              qkv_unit(8, 0), qkv_unit(8, 1)):
                        u()
                    for a in range(8):
                        others = []
                        if a + 1 < 8:
                            others += [qkv_unit(a + 1, 0), qkv_unit(a + 1, 1),
                                       qkv_unit(9 + a, 0), qkv_unit(9 + a, 1)]
                        if a == 0:
                            others += [vproj_unit(m, n)
                                       for m in range(8) for n in range(2)]
                        if a >= 1:
                            hp = 2 * (a - 1)
                            others += [ctx_unit(hp, 0), ctx_unit(hp, 1),
                                       ctx_unit(hp + 1, 0), ctx_unit(hp + 1, 1)]
                        if a >= 2:
                            others += [p4_unit(a - 2, 0), p4_unit(a - 2, 1)]
                        sts = sc_pair_units(a)
                        ns = len(sts)
                        no = len(others)
                        for idx, su in enumerate(sts):
                            su()
                            for u in others[
                                (idx * no) // ns:((idx + 1) * no) // ns
                            ]:
                                u()
                    for u in (ctx_unit(H - 2, 0), ctx_unit(H - 2, 1),
                              ctx_unit(H - 1, 0), ctx_unit(H - 1, 1),
                              p4_unit(6, 0), p4_unit(6, 1),
                              p4_unit(7, 0), p4_unit(7, 1)):
                        u()

                # ---- out-proj + residual + LN1 + h^T + FFN1 overlap ----
                # Residual rides the PE: the out-proj PSUM group starts with
                # identb.T @ (512*x)  (bf16), then accumulates the fp8 DR
                # out-proj matmuls.  LN is scale-invariant, so the 512x
                # scale needs no unscale; bn_stats reads the PSUM directly.
                with (
                    tc.tile_pool(name="p_ff1", bufs=1) as p_ff1,
                    tc.tile_pool(name="p_w2r", bufs=1) as p_w2r,
                ):
                  ff1 = p_ff1.tile([128, JT, T], BF16)
                  w2sb = p_w2r.tile([128, JT, C], BF16)

                  def w2_dma(jc):
                      nc.gpsimd.dma_start(
                          out=w2sb[:, jc * 4:(jc + 1) * 4, :],
                          in_=w2_d[:, jc * 4:(jc + 1) * 4, :],
                      )

                  with (
                    tc.tile_pool(name="pp_ao", bufs=2, space="PSUM") as pp_ao,
                    tc.tile_pool(name="pp_f1", bufs=3, space="PSUM") as pp_f1,
                    tc.tile_pool(name="pp_tr7", bufs=1, space="PSUM") as pp_tr7,
                  ):
                    def op_ln1(m):
                        ao = pp_ao.tile([128, C], F32, tag="ao")
                        for n in range(2):
                            nc.tensor.matmul(
                                ao[:, n * 512:(n + 1) * 512],
                                lhsT=identb,
                                rhs=xts[m][:, n * 512:(n + 1) * 512],
                                start=True, stop=False,
                            )
                            for ks in range(4):
                                nc.tensor.matmul(
                                    ao[:, n * 512:(n + 1) * 512],
                                    lhsT=ctxT8[
                                        :, 2 * ks:2 * ks + 2,
                                        m * 128:(m + 1) * 128,
                                    ],
                                    rhs=wo8[
                                        :, 2 * ks:2 * ks + 2,
                                        n * 512:(n + 1) * 512,
                                    ],
                                    start=False, stop=(ks == 3),
                                    perf_mode=PM.DoubleRow,
                                )
                        layernorm(ao, hbf[:, m, :], g1bc, b1lnbc, p_ln1)

                    def ht_half(n):
                        for b in range(KT):
                            tr = pp_tr7.tile([128, 512], BF16, tag="tr7")
                            for a in range(4):
                                nc.tensor.transpose(
                                    tr[:, a * 128:(a + 1) * 128],
                                    hbf[:, n * 4 + a, b * 128:(b + 1) * 128],
                                    identb,
                                )
                            if b < 6:
                                nc.vector.tensor_scalar_mul(
                                    hT8[:, b, n * 512:(n + 1) * 512], tr, 8.0
                                )
                            else:
                                nc.vector.tensor_scalar_mul(
                                    hTb[:, b - 6, n * 512:(n + 1) * 512],
                                    tr, 16.0,
                                )

                    def f1_half(j, n, dve_relu=False):
                        ps = pp_f1.tile([128, 512], F32, tag="f1h")
                        w8c, wtc = w1cs[j // 4]
                        jj = j % 4
                        for p in range(3):
                            nc.tensor.matmul(
                                ps,
                                lhsT=w8c[:, p, :, jj, :],
                                rhs=hT8[:, 2 * p:2 * p + 2,
                                        n * 512:(n + 1) * 512],
                                start=(p == 0), stop=False,
                                perf_mode=PM.DoubleRow,
                            )
                        for k in range(2):
                            nc.tensor.matmul(
                                ps,
                                lhsT=wtc[:, k, jj, :],
                                rhs=hTb[:, k, n * 512:(n + 1) * 512],
                                start=False, stop=(k == 1),
                            )
                        if dve_relu and not has_b1:
                            # LN1 window: ACT does the LN chain and DVE the
                            # stats there; run these fillers' ReLU on the
                            # idle GpSimd so the f1 PSUM slots recycle
                            # without delaying either chain engine
                            nc.vector.tensor_scalar(
                                out=ff1[:, j, n * 512:(n + 1) * 512],
                                in0=ps, scalar1=1.0 / 256.0, scalar2=0.0,
                                op0=OP.mult, op1=OP.max,
                            )
                            return
                        bias = b1sb[:, j:j + 1] if has_b1 else 0.0
                        nc.scalar.activation(
                            ff1[:, j, n * 512:(n + 1) * 512], ps,
                            AF.Relu, bias=bias, scale=1.0 / 256.0,
                        )

                    for m in range(MT):
                        if m + 2 < MT:
                            xt_dma(m + 2)
                        op_ln1(m)
                        if m == 3:
                            ht_half(0)
                        elif m >= 4:
                            f1_half(m - 4, 0, dve_relu=True)
                    for j in range(4, 16):
                        f1_half(j, 0, dve_relu=True)
                    ht_half(1)
                    w2_dma(0)
                    for jc in range(4):
                        for jj in range(4):
                            f1_half(jc * 4 + jj, 1)
                        w1_dma(4 + jc)
                        w2_dma(1 + jc)
                    for jc in range(4, 8):
                        if jc < 7:
                            w2_dma(jc + 1)
                        for jj in range(4):
                            j = jc * 4 + jj
                            f1_half(j, 0)
                            f1_half(j, 1)

                  # ---- FFN2 + residual + LN2 + out^T ----
                  with (
                    tc.tile_pool(name="p_ln2", bufs=2) as p_ln2,
                    tc.tile_pool(name="p_z2", bufs=2) as p_z2,
                    tc.tile_pool(name="p_out", bufs=4) as p_out,
                    tc.tile_pool(name="pp_f2", bufs=1, space="PSUM") as pp_f2,
                    tc.tile_pool(name="pp_t11", bufs=2, space="PSUM") as pp_t11,
                  ):
                    def ffn2_m(m):
                        ps = pp_f2.tile([128, C], F32, tag=f"f2_{m % 2}",
                                        name=f"f2ps_{m}")
                        for n in range(2):
                            nc.tensor.matmul(
                                ps[:, n * 512:(n + 1) * 512],
                                lhsT=identb,
                                rhs=hbf[:, m, n * 512:(n + 1) * 512],
                                start=True, stop=False,
                            )
                            for j in range(JT):
                                nc.tensor.matmul(
                                    ps[:, n * 512:(n + 1) * 512],
                                    lhsT=ff1[:, j, m * 128:(m + 1) * 128],
                                    rhs=w2sb[:, j, n * 512:(n + 1) * 512],
                                    start=False, stop=(j == JT - 1),
                                )
                        return ps

                    def ln2_m(m, ps):
                        if has_b2:
                            resid2 = p_ln2.tile([128, C], F32, tag="resid2")
                            nc.vector.tensor_add(resid2, ps, b2bc)
                            src = resid2
                        else:
                            src = ps
                        zt = p_z2.tile([128, C], BF16, tag="z2",
                                       name=f"z2_{m}")
                        layernorm(src, zt, g2bc, b2lnbc, p_ln2)
                        return zt

                    def outT(m, zt):
                        # bf16 transposes (final LN2 output; ~0.1% rounding)
                        for b in range(KT):
                            tr = pp_t11.tile([128, 128], BF16, tag="t11")
                            nc.tensor.transpose(
                                tr, zt[:, b * 128:(b + 1) * 128], identb,
                            )
                            osb = p_out.tile([128, 128], F32, tag="osb")
                            nc.vector.tensor_copy(osb, tr)
                            eng = nc.sync if b % 2 == 0 else nc.scalar
                            eng.dma_start(
                                out=out_d[
                                    b * 128:(b + 1) * 128,
                                    m * 128:(m + 1) * 128,
                                ],
                                in_=osb,
                            )

                    for m in range(MT):
                        ps = ffn2_m(m)
                        outT(m, ln2_m(m, ps))
    _split_multiwait(nc)
    return nc


_prog_cache = {}


def prepare(
    x, in_proj_w, out_proj_w, ln1_g, ln1_b, ln2_g, ln2_b, w1, b1, w2, b2
):
    """Returns (nc, in_maps): the built program plus per-core input maps."""
    x = np.asarray(x, dtype=np.float32)
    in_proj_w = np.asarray(in_proj_w, dtype=np.float32)
    out_proj_w = np.asarray(out_proj_w, dtype=np.float32)
    ln1_g = np.asarray(ln1_g, dtype=np.float32)
    ln1_b = np.asarray(ln1_b, dtype=np.float32)
    ln2_g = np.asarray(ln2_g, dtype=np.float32)
    ln2_b = np.asarray(ln2_b, dtype=np.float32)
    w1 = np.asarray(w1, dtype=np.float32)
    b1 = np.asarray(b1, dtype=np.float32)
    w2 = np.asarray(w2, dtype=np.float32)
    b2 = np.asarray(b2, dtype=np.float32)

    flags = (
        not np.all(ln1_g == 1.0),
        not np.all(ln1_b == 0.0),
        not np.all(ln2_g == 1.0),
        not np.all(ln2_b == 0.0),
        not np.all(b1 == 0.0),
        not np.all(b2 == 0.0),
    )
    if flags not in _prog_cache:
        _prog_cache[flags] = _build(flags)
    nc = _prog_cache[flags]

    peT, ident, mask01 = _host_constants()

    winT = np.ascontiguousarray(in_proj_w.T)             # [C, 3C] f32
    wqk = np.ascontiguousarray(np.transpose(
        (winT[:, :2 * C] * SW).reshape(KT, 128, 16, 128), (1, 2, 0, 3)
    )).astype(E4)                                        # [128, 16, KT, 128]
    wv = np.ascontiguousarray(np.transpose(
        (winT[:, 2 * C:] * SW).reshape(KT, 128, C), (1, 0, 2)
    )).astype(E4)                                        # [128, KT, C]
    wo = np.ascontiguousarray(np.transpose(
        (out_proj_w.T * SWO).reshape(KT, 128, C), (1, 0, 2)
    )).astype(E4)                                        # [128, KT, C]
    w1r = np.transpose(
        w1.T.reshape(KT, 128, JT, 128), (1, 0, 2, 3)
    )                                                    # [128, KT, JT, 128]
    w18 = np.ascontiguousarray(
        (w1r[:, 0:6] * 32.0).reshape(128, 3, 2, JT, 128)
    ).astype(E4)                                         # [128, 3, 2, JT, 128]
    w1t = np.ascontiguousarray(w1r[:, 6:8] * 16.0).astype(BF)
    w2b = np.ascontiguousarray(np.transpose(
        w2.T.reshape(JT, 128, C), (1, 0, 2)
    )).astype(BF)                                        # [128, JT, C]

    shared = {
        "wqk8": wqk,
        "wv8": wv,
        "wo8": wo,
        "w18": w18,
        "w1t": w1t,
        "w2b": w2b,
        "identf": ident.astype(np.float32),
        "identb": ident.astype(BF),
        "mask01": mask01,
    }
    if flags[0]:
        shared["g1"] = ln1_g
    if flags[1]:
        shared["b1ln"] = ln1_b
    if flags[2]:
        shared["g2"] = ln2_g
    if flags[3]:
        shared["b2ln"] = ln2_b
    if flags[4]:
        shared["b1t"] = np.ascontiguousarray(b1.reshape(JT, 128).T)
    if flags[5]:
        shared["b2"] = b2

    in_maps = []
    for i in range(N):
        m = dict(shared)
        xq = (x[i] + peT) * SX                            # [C, T]
        m["xq8"] = np.ascontiguousarray(
            np.transpose(xq.reshape(KT, 128, T), (1, 0, 2))
        ).astype(E4)                                      # [128, KT, T]
        # residual ships pre-scaled by SV*SWO (the attention-psum scale;
        # LN is scale-invariant) so it can ride the out-proj matmul group
        m["x_tc"] = np.ascontiguousarray(
            x[i].T * (SV * SWO)
        ).astype(BF)                                      # [T, C] bf16
        in_maps.append(m)
    return nc, in_maps


def kernel(**inputs):
    nc, in_maps = prepare(**inputs)
    res = run_bass_kernel_spmd(nc, in_maps, core_ids=list(range(N)))
    out = np.stack([res.results[i]["out"] for i in range(N)], axis=0)
    return out.astype(np.float32)



# revision 58
# speedup vs baseline: 1.1761x; 1.0459x over previous
"""Trainium2 Bass kernel for nn_MhaSelfAttenLayer (dense transformer layer).

Data-parallel over batch: each of the 8 NeuronCores runs the full layer on
one batch element. No collectives.

Precision plan (validated numerically; sim matches HW to ~1e-4):
- Attention path (QKV proj, V, out proj, attn*V) in fp8 e4m3 with DoubleRow
  matmuls; scores in bf16 (64-wide contraction gets no DoubleRow benefit);
  fp32 PSUM accumulation everywhere.
- FFN1 contracts 6 of 8 c-tiles in fp8 DoubleRow (x32/x8 scales) and the
  last 2 in bf16 (x16/x16, so both partials carry the same 256x PSUM
  scale, removed in the ReLU activation scale).  7/8 or 8/8 fp8 breaches
  the 2e-2 budget; 6/8 lands at 1.89e-2.  FFN2 stays bf16.
- Residual adds ride the PE: each out-proj / FFN2 PSUM accumulation group
  starts with identity.T @ residual (bf16), and the LayerNorms read stats
  straight from PSUM.  LN is scale-invariant, so the fp8 512x scale on the
  attention PSUM needs no unscale; x ships pre-scaled as bf16.
- Scale folding: qkT holds 256*q ; the 256^2 factor is removed inside the
  exp() activation scale (exact power of two), v/ctx/out-proj scales fold
  into existing copies, so fp8 costs no extra instructions.
"""

import math

import numpy as np
import ml_dtypes

import concourse.bass as bass
import concourse.tile as tile
from concourse import mybir
from concourse.bass_utils import run_bass_kernel_spmd
from concourse.vector_clock import ScopedClock, VectorClock

F32 = mybir.dt.float32
BF16 = mybir.dt.bfloat16
F8 = mybir.dt.float8e4
BF = ml_dtypes.bfloat16
E4 = ml_dtypes.float8_e4m3

N, T, C, H, HD, HID = 8, 1024, 1024, 16, 64, 4096
KT = C // 128           # 8 c-tiles
MT = T // 128           # 8 t-tiles
JT = HID // 128         # 32 hid-tiles
EPS = 1e-5
AF = mybir.ActivationFunctionType
OP = mybir.AluOpType
PM = mybir.MatmulPerfMode

SX = 8.0                # xq8 = SX*(x+pe)
SW = 32.0               # wqk8 = SW*w  -> qkT = 256*q
ESC = 1.0 / (256.0 * 256.0 * 8.0)   # exp arg unscale (2^-19), incl 1/sqrt(hd)
SE = 2.0                # ex = SE*exp(score)
SV = 16.0               # v8 = SV*v ; ctxT8 = SV*ctx
SWO = 32.0              # wo8 = SWO*wo
AOS = 1.0 / (SV * SWO)  # attn-out unscale

_patched = False


def _patch_drain():
    """This walrus build accepts at most 1 sem wait per instruction (2 for
    EventSemaphore). Tile's final drain packs every outstanding proc wait
    onto a single drain -> codegen error. Emit one drain per proc instead."""
    global _patched
    if _patched:
        return
    _patched = True

    def _split_drain_and_barrier(self, tick_clock, wait_clock):
        gclock = tick_clock.global_clock
        n = len(gclock)
        for proc in range(n):
            t = gclock[proc]
            if t <= 0:
                continue
            vc = VectorClock([0] * n)
            vc.require_at_least(proc, t)
            d = self.nc.sync.drain()
            wait_clock.add_sem_waits(d.ins, ScopedClock({None: vc}))
        self.nc.all_engine_barrier()
        popped = self.nc._tile_sem_poison_stack.pop()
        assert popped is self._sem_poison
        self.nc.clear_and_free_semaphores(list(self.sems.allocated().values()))
        self.nc.all_engine_barrier()

    tile.TileContext._drain_and_barrier = _split_drain_and_barrier


def _split_multiwait(nc):
    """This walrus build accepts at most one sem wait per instruction. Hoist
    excess waits onto freshly created same-engine nops placed immediately
    before the over-limit instruction (engine streams run in order, so the
    nop blocking first preserves the dependency)."""
    import bass_rust

    engmap = {
        mybir.EngineType.PE: nc.tensor,
        mybir.EngineType.DVE: nc.vector,
        mybir.EngineType.Activation: nc.scalar,
        mybir.EngineType.SP: nc.sync,
        mybir.EngineType.Pool: nc.gpsimd,
    }
    blocks = list(nc.main_func.blocks)
    records = []
    for bi, bb in enumerate(blocks):
        for ins in bb.instructions:
            si = ins.sync_info
            if si is None or not si.on_wait:
                continue
            waits = list(si.on_wait)
            limit = 2 if type(ins).__name__ == "InstEventSemaphore" else 1
            if len(waits) > limit:
                records.append((ins.name, ins, waits[:-limit]))
                si.on_wait = waits[-limit:]
    if not records:
        return
    carriers = {}
    nop_names = set()
    for name, ins, excess in records:
        lst = []
        for w in excess:
            nb = engmap[ins.engine].nop()
            nb.ins.sync_info = bass_rust.SyncInfo(on_wait=[w], on_update=[])
            nop_names.add(nb.ins.name)
            lst.append(nb.ins)
        carriers[name] = lst
    for bb in blocks:
        il = list(bb.instructions)
        out = []
        changed = False
        for ins in il:
            if ins.name in nop_names:
                changed = True
                continue
            if ins.name in carriers:
                out.extend(carriers[ins.name])
                changed = True
            out.append(ins)
        if changed:
            bb.instructions = out


def _host_constants():
    pos = np.arange(T, dtype=np.float32)[:, None]
    div = np.exp(
        np.arange(0, C, 2, dtype=np.float32) * (-math.log(10000.0) / C)
    )
    ang = pos * div
    pe = np.stack([np.sin(ang), np.cos(ang)], axis=-1).reshape(T, C)
    peT = np.ascontiguousarray(pe.T)                    # [C, T]

    ident = np.eye(128, dtype=np.float32)
    kk = np.arange(128)
    # mask01[k, q] = 0 where q < k (future key within diagonal block)
    mask01 = np.where(kk[None, :] < kk[:, None], 0.0, 1.0).astype(E4)
    return peT, ident, mask01


def _build(flags):
    """flags = (g1, b1ln, g2, b2ln, b1, b2) booleans for non-trivial params."""
    has_g1, has_b1ln, has_g2, has_b2ln, has_b1, has_b2 = flags
    _patch_drain()
    nc = bass.Bass(trn_type="TRN2")

    # ---- DRAM I/O ----
    xq_d = nc.dram_tensor("xq8", [128, KT, T], F8, kind="ExternalInput")
    x_tc = nc.dram_tensor("x_tc", [T, C], BF16, kind="ExternalInput")
    wqk_d = nc.dram_tensor("wqk8", [128, 16, KT, 128], F8, kind="ExternalInput")
    wv_d = nc.dram_tensor("wv8", [128, KT, C], F8, kind="ExternalInput")
    wo_d = nc.dram_tensor("wo8", [128, KT, C], F8, kind="ExternalInput")
    w18_d = nc.dram_tensor("w18", [128, 3, 2, JT, 128], F8,
                           kind="ExternalInput")
    w1t_d = nc.dram_tensor("w1t", [128, 2, JT, 128], BF16,
                           kind="ExternalInput")
    w2_d = nc.dram_tensor("w2b", [128, JT, C], BF16, kind="ExternalInput")
    idb_d = nc.dram_tensor("identb", [128, 128], BF16, kind="ExternalInput")
    mk_d = nc.dram_tensor("mask01", [128, 128], F8, kind="ExternalInput")
    if has_g1:
        g1_d = nc.dram_tensor("g1", [C], F32, kind="ExternalInput")
    if has_b1ln:
        b1ln_d = nc.dram_tensor("b1ln", [C], F32, kind="ExternalInput")
    if has_g2:
        g2_d = nc.dram_tensor("g2", [C], F32, kind="ExternalInput")
    if has_b2ln:
        b2ln_d = nc.dram_tensor("b2ln", [C], F32, kind="ExternalInput")
    if has_b1:
        b1_d = nc.dram_tensor("b1t", [128, JT], F32, kind="ExternalInput")
    if has_b2:
        b2_d = nc.dram_tensor("b2", [C], F32, kind="ExternalInput")
    out_d = nc.dram_tensor("out", [C, T], F32, kind="ExternalOutput")

    def bcast_ap(dram_1d, n):
        return bass.AP(tensor=dram_1d.tensor, offset=0, ap=[[0, 128], [1, n]])

    with tile.TileContext(nc) as tc:
        with (
            tc.tile_pool(name="consts", bufs=1) as consts,
            tc.tile_pool(name="smalls", bufs=12) as smalls,
            tc.tile_pool(name="p_hbf", bufs=1) as p_hbf,
            tc.tile_pool(name="p_hT", bufs=1) as p_hT,
        ):
            # ---- constants (ACT-ring DMAs; SP ring is kept clear for the
            # latency-critical xq8/wqk loads) ----
            zbias = consts.tile([128, 1], F32)
            nc.vector.memset(zbias, 0.0)
            nc.const_aps.aps[(F32, 0.0)] = zbias
            epsb = consts.tile([128, 1], F32)
            nc.vector.memset(epsb, EPS)
            lnb = consts.tile([128, 1], F32)
            nc.vector.memset(lnb, float(math.log(SE)))
            warm = consts.tile([128, 64], BF16)
            nc.vector.memset(warm, 0.25)
            tblw = smalls.tile([128, 1], F32, tag="tblw")
            nc.scalar.activation(tblw, epsb, AF.Exp, bias=0.0, scale=1.0)
            nc.scalar.activation(tblw, epsb, AF.Sqrt, bias=0.0, scale=1.0)
            mask01 = consts.tile([128, 128], F8)
            nc.gpsimd.dma_start(out=mask01, in_=mk_d[:, :])
            identb = consts.tile([128, 128], BF16)
            nc.gpsimd.dma_start(out=identb, in_=idb_d[:, :])
            g1bc = b1lnbc = g2bc = b2lnbc = b1sb = b2bc = None
            if has_g1:
                g1bc = consts.tile([128, C], F32)
                nc.scalar.dma_start(out=g1bc, in_=bcast_ap(g1_d, C))
            if has_b1ln:
                b1lnbc = consts.tile([128, C], F32)
                nc.scalar.dma_start(out=b1lnbc, in_=bcast_ap(b1ln_d, C))
            if has_g2:
                g2bc = consts.tile([128, C], F32)
                nc.scalar.dma_start(out=g2bc, in_=bcast_ap(g2_d, C))
            if has_b2ln:
                b2lnbc = consts.tile([128, C], F32)
                nc.scalar.dma_start(out=b2lnbc, in_=bcast_ap(b2ln_d, C))
            if has_b1:
                b1sb = consts.tile([128, JT], F32)
                nc.scalar.dma_start(out=b1sb, in_=b1_d[:, :])
            if has_b2:
                b2bc = consts.tile([128, C], F32)
                nc.scalar.dma_start(out=b2bc, in_=bcast_ap(b2_d, C))

            # ---- PE warm-up: ~6us of tiny matmuls during the input-DMA
            # wait so the HAM clock-gate reaches K=8/8 before real work ----
            with tc.tile_pool(name="pp_warm", bufs=1, space="PSUM") as pp_w:
                wps = pp_w.tile([64, 64], F32)
                for _ in range(84):
                    nc.tensor.matmul(wps, lhsT=warm[:, 0:64],
                                     rhs=warm[:, 0:64], start=True, stop=True)

            hbf = p_hbf.tile([128, MT, C], BF16)
            hT8 = p_hT.tile([128, 6, T], F8)
            hTb = p_hT.tile([128, 2, T], BF16)

            def layernorm(resid, out_tile, gbc, bbc, zpool):
                stats = smalls.tile([128, 2, 6], F32, tag="stats")
                nc.vector.bn_stats(out=stats[:, 0, :], in_=resid[:, 0:512])
                nc.vector.bn_stats(out=stats[:, 1, :], in_=resid[:, 512:1024])
                mv = smalls.tile([128, 2], F32, tag="mv")
                nc.vector.bn_aggr(out=mv, in_=stats)
                std = smalls.tile([128, 1], F32, tag="std")
                nc.scalar.activation(std, mv[:, 1:2], AF.Sqrt, bias=epsb)
                istd = smalls.tile([128, 1], F32, tag="istd")
                nc.vector.reciprocal(istd, std)
                nbias = smalls.tile([128, 1], F32, tag="nbias")
                nc.vector.tensor_scalar(
                    out=nbias, in0=mv[:, 0:1], scalar1=istd, scalar2=-1.0,
                    op0=OP.mult, op1=OP.mult,
                )
                if gbc is None and bbc is None:
                    # two halves: downstream per-128-col transposes unblock
                    # after half 0 (subtile deps), hiding the chain latency
                    nc.scalar.activation(
                        out_tile[:, 0:512], resid[:, 0:512],
                        AF.Identity, bias=nbias, scale=istd,
                    )
                    nc.scalar.activation(
                        out_tile[:, 512:1024], resid[:, 512:1024],
                        AF.Identity, bias=nbias, scale=istd,
                    )
                else:
                    z = zpool.tile([128, C], F32, tag="zln")
                    nc.vector.tensor_scalar(
                        out=z, in0=resid, scalar1=istd, scalar2=nbias,
                        op0=OP.mult, op1=OP.add,
                    )
                    if gbc is not None:
                        nc.vector.tensor_mul(z, z, gbc)
                    if bbc is not None:
                        nc.vector.tensor_add(z, z, bbc)
                    nc.vector.tensor_copy(out_tile, z)

            # ================= front: QKV + attention =================
            with (
                tc.tile_pool(name="p_ctxT", bufs=1) as p_ctxT,
                tc.tile_pool(name="p_ln1", bufs=3) as p_ln1,
                tc.tile_pool(name="p_w1", bufs=4) as p_w1,
            ):
                ctxT8 = p_ctxT.tile([128, KT, T], F8)
                wo8 = p_ctxT.tile([128, KT, C], F8)
                # xt / w1 pools live OUTSIDE the attention pools so their
                # DMAs have no address-reuse deps and land during attention
                xts = {}

                def xt_dma(m):
                    xt = p_ln1.tile([128, C], BF16, tag="xt",
                                    name=f"xt_{m}")
                    nc.sync.dma_start(
                        out=xt, in_=x_tc[m * 128:(m + 1) * 128, :]
                    )
                    xts[m] = xt

                w1cs = {}

                def w1_dma(jc):
                    w8c = p_w1.tile([128, 3, 2, 4, 128], F8, tag="w8c",
                                    name=f"w8c_{jc}")
                    nc.gpsimd.dma_start(
                        out=w8c, in_=w18_d[:, :, :, jc * 4:(jc + 1) * 4, :]
                    )
                    wtc = p_w1.tile([128, 2, 4, 128], BF16, tag="wtc",
                                    name=f"wtc_{jc}")
                    nc.gpsimd.dma_start(
                        out=wtc, in_=w1t_d[:, :, jc * 4:(jc + 1) * 4, :]
                    )
                    w1cs[jc] = (w8c, wtc)

                with (
                    tc.tile_pool(name="p_att", bufs=1) as p_att,
                    tc.tile_pool(name="p_ex", bufs=4) as p_ex,
                    tc.tile_pool(name="pp_big", bufs=6, space="PSUM") as pp_big,
                    tc.tile_pool(name="pp_ctx", bufs=1, space="PSUM") as pp_ctx,
                    tc.tile_pool(name="pp_tr", bufs=1, space="PSUM") as pp_tr,
                ):
                    qkT = p_att.tile([128, 16, T], BF16)
                    vsb = p_att.tile([128, MT, H * (HD + 1)], F8)
                    v4 = vsb.rearrange("p m (h e) -> p m h e", h=H)
                    ctxf = p_att.tile([128, MT, C], BF16)
                    xq8 = p_att.tile([128, KT, T], F8)
                    wqk8 = p_att.tile([128, 16, KT, 128], F8)
                    wv8 = p_att.tile([128, KT, C], F8)

                    nc.sync.dma_start(out=xq8[:, 0:2, :],
                                      in_=xq_d[:, 0:2, :])
                    nc.scalar.dma_start(out=xq8[:, 2:4, :],
                                      in_=xq_d[:, 2:4, :])
                    nc.gpsimd.dma_start(out=xq8[:, 4:6, :],
                                        in_=xq_d[:, 4:6, :])
                    nc.scalar.dma_start(out=xq8[:, 6:8, :],
                                      in_=xq_d[:, 6:8, :])
                    nc.sync.dma_start(out=wqk8[:, 0, :, :],
                                      in_=wqk_d[:, 0, :, :])
                    nc.sync.dma_start(out=wqk8[:, 8, :, :],
                                      in_=wqk_d[:, 8, :, :])
                    for lo, hi in ((1, 3), (9, 11), (3, 6), (11, 14),
                                   (6, 8), (14, 16)):
                        nc.sync.dma_start(out=wqk8[:, lo:hi, :, :],
                                          in_=wqk_d[:, lo:hi, :, :])
                    nc.scalar.dma_start(out=wv8, in_=wv_d[:, :, :])
                    nc.gpsimd.dma_start(out=wo8, in_=wo_d[:, :, :])
                    w1_dma(0)
                    w1_dma(1)
                    w1_dma(2)
                    w1_dma(3)
                    xt_dma(0)
                    xt_dma(1)
                    nc.vector.memset(v4[:, :, :, HD:HD + 1], 1.0)

                    def qkv_unit(m, n):
                        def u():
                            mm = pp_big.tile([128, 512], F32, tag="big")
                            for ks in range(4):
                                nc.tensor.matmul(
                                    mm,
                                    lhsT=wqk8[:, m, 2 * ks:2 * ks + 2, :],
                                    rhs=xq8[
                                        :, 2 * ks:2 * ks + 2,
                                        n * 512:(n + 1) * 512,
                                    ],
                                    start=(ks == 0), stop=(ks == 3),
                                    perf_mode=PM.DoubleRow,
                                )
                            nc.vector.tensor_copy(
                                qkT[:, m, n * 512:(n + 1) * 512], mm
                            )
                        return u

                    def vproj_unit(m, n):
                        def u():
                            mm = pp_big.tile([128, 512], F32, tag="big")
                            for ks in range(4):
                                nc.tensor.matmul(
                                    mm,
                                    lhsT=xq8[
                                        :, 2 * ks:2 * ks + 2,
                                        m * 128:(m + 1) * 128,
                                    ],
                                    rhs=wv8[
                                        :, 2 * ks:2 * ks + 2,
                                        n * 512:(n + 1) * 512,
                                    ],
                                    start=(ks == 0), stop=(ks == 3),
                                    perf_mode=PM.DoubleRow,
                                )
                            nc.vector.tensor_scalar_mul(
                                v4[:, m, n * 8:(n + 1) * 8, 0:HD],
                                mm.rearrange("p (h e) -> p h e", h=8),
                                1.0 / SV,
                            )
                        return u

                    exmap = {}

                    def sc_pair_units(a):
                        """Score+exp units for head pair (2a, 2a+1).  The even
                        head's kt/qt live on partitions 0-63, the odd head's on
                        64-127, so the two back-to-back matmuls land on disjoint
                        PE row groups (tile_position auto-derives) and execute
                        concurrently — 2x score throughput."""
                        he, ho = 2 * a, 2 * a + 1
                        exe = p_ex.tile([128, MT, T], F8, tag="ex",
                                        name=f"ex_{he}")
                        exo = p_ex.tile([128, MT, T], F8, tag="ex",
                                        name=f"ex_{ho}")
                        exmap[he], exmap[ho] = exe, exo
                        units = []
                        for j in range(MT):
                            qspan = T - j * 128
                            for lo in range(0, qspan, 512):
                                hi = min(lo + 512, qspan)

                                def u(j=j, lo=lo, hi=hi):
                                    qlo, qhi = j * 128 + lo, j * 128 + hi
                                    ste = pp_big.tile(
                                        [128, 512], F32, tag="big"
                                    )
                                    sto = pp_big.tile(
                                        [128, 512], F32, tag="big"
                                    )
                                    nc.tensor.matmul(
                                        ste[:, 0:hi - lo],
                                        lhsT=qkT[0:64, 8 + a,
                                                 j * 128:(j + 1) * 128],
                                        rhs=qkT[0:64, a, qlo:qhi],
                                        start=True, stop=True,
                                    )
                                    nc.tensor.matmul(
                                        sto[:, 0:hi - lo],
                                        lhsT=qkT[64:128, 8 + a,
                                                 j * 128:(j + 1) * 128],
                                        rhs=qkT[64:128, a, qlo:qhi],
                                        start=True, stop=True,
                                    )
                                    for exh, st in ((exe, ste), (exo, sto)):
                                        nc.scalar.activation(
                                            exh[:, j, qlo:qhi],
                                            st[:, 0:hi - lo],
                                            AF.Exp, bias=lnb, scale=ESC,
                                        )
                                        if lo == 0:
                                            nc.gpsimd.tensor_mul(
                                                exh[:, j,
                                                    j * 128:(j + 1) * 128],
                                                exh[:, j,
                                                    j * 128:(j + 1) * 128],
                                                mask01,
                                            )
                                units.append(u)
                        return units

                    def ctx_unit(h, i2):
                        exh = exmap[h]

                        def u():
                            # normal-mode fp8: 128-col weight loads keep the
                            # compiler's fast-weight-load (DoubleRow at free
                            # dim 65 is LDWEIGHTS-bound and congests the
                            # weight port for qkv/score loads)
                            cps = pp_ctx.tile([128, 4, HD + 1], F32, tag="cps")
                            for ii in range(4):
                                i = i2 * 4 + ii
                                for j in range(i + 1):
                                    nc.tensor.matmul(
                                        cps[:, ii, :],
                                        lhsT=exh[:, j, i * 128:(i + 1) * 128],
                                        rhs=v4[:, j, h, :],
                                        start=(j == 0), stop=(j == i),
                                    )
                            rden = smalls.tile([128, 4], F32, tag="rden")
                            nc.vector.reciprocal(
                                rden,
                                cps.rearrange("p i e -> p (i e)")[:, HD::HD + 1],
                            )
                            rb = bass.AP(
                                tensor=rden.tensor, offset=rden.offset,
                                ap=[rden.ap[0], [rden.ap[1][0], 4], [0, HD]],
                            )
                            nc.vector.tensor_mul(
                                ctxf[:, i2 * 4:(i2 + 1) * 4,
                                     h * HD:(h + 1) * HD],
                                cps[:, :, 0:HD], rb,
                            )
                        return u

                    def p4_unit(b, n):
                        def u():
                            tr = pp_tr.tile([128, 512], BF16, tag="tr")
                            for a in range(4):
                                nc.tensor.transpose(
                                    tr[:, a * 128:(a + 1) * 128],
                                    ctxf[:, n * 4 + a, b * 128:(b + 1) * 128],
                                    identb,
                                )
                            nc.vector.tensor_copy(
                                ctxT8[:, b, n * 512:(n + 1) * 512], tr
                            )
                        return u

                    # --- software-pipelined emission: paired score chunks
                    # (2x concurrent via row tiling) are spread uniformly
                    # between other PE work so the exp stream never starves;
                    # qkT pairs are produced one slot ahead. ---
                    for u in (qkv_unit(0, 0), qkv_unit(0, 1),
                              qkv_unit(8, 0), qkv_unit(8, 1)):
                        u()
                    for a in range(8):
                        others = []
                        if a + 1 < 8:
                            others += [qkv_unit(a + 1, 0), qkv_unit(a + 1, 1),
                                       qkv_unit(9 + a, 0), qkv_unit(9 + a, 1)]
                        if a == 0:
                            others += [vproj_unit(m, n)
                                       for m in range(8) for n in range(2)]
                        if a >= 1:
                            hp = 2 * (a - 1)
                            others += [ctx_unit(hp, 0), ctx_unit(hp, 1),
                                       ctx_unit(hp + 1, 0), ctx_unit(hp + 1, 1)]
                        if a >= 2:
                            others += [p4_unit(a - 2, 0), p4_unit(a - 2, 1)]
                        sts = sc_pair_units(a)
                        ns = len(sts)
                        no = len(others)
                        for idx, su in enumerate(sts):
                            su()
                            for u in others[
                                (idx * no) // ns:((idx + 1) * no) // ns
                            ]:
                                u()
                    for u in (ctx_unit(H - 2, 0), ctx_unit(H - 2, 1),
                              ctx_unit(H - 1, 0), ctx_unit(H - 1, 1),
                              p4_unit(6, 0), p4_unit(6, 1),
                              p4_unit(7, 0), p4_unit(7, 1)):
                        u()

                # ---- out-proj + residual + LN1 + h^T + FFN1 overlap ----
                # Residual rides the PE: the out-proj PSUM group starts with
                # identb.T @ (512*x)  (bf16), then accumulates the fp8 DR
                # out-proj matmuls.  LN is scale-invariant, so the 512x
                # scale needs no unscale; bn_stats reads the PSUM directly.
                with (
                    tc.tile_pool(name="p_ff1", bufs=1) as p_ff1,
                    tc.tile_pool(name="p_w2r", bufs=1) as p_w2r,
                ):
                  ff1 = p_ff1.tile([128, JT, T], BF16)
                  w2sb = p_w2r.tile([128, JT, C], BF16)

                  def w2_dma(jc):
                      nc.gpsimd.dma_start(
                          out=w2sb[:, jc * 4:(jc + 1) * 4, :],
                          in_=w2_d[:, jc * 4:(jc + 1) * 4, :],
                      )

                  with (
                    tc.tile_pool(name="pp_ao", bufs=2, space="PSUM") as pp_ao,
                    tc.tile_pool(name="pp_f1", bufs=3, space="PSUM") as pp_f1,
                    tc.tile_pool(name="pp_tr7", bufs=1, space="PSUM") as pp_tr7,
                  ):
                    def op_ln1(m):
                        ao = pp_ao.tile([128, C], F32, tag="ao")
                        for n in range(2):
                            nc.tensor.matmul(
                                ao[:, n * 512:(n + 1) * 512],
                                lhsT=identb,
                                rhs=xts[m][:, n * 512:(n + 1) * 512],
                                start=True, stop=False,
                            )
                            for ks in range(4):
                                nc.tensor.matmul(
                                    ao[:, n * 512:(n + 1) * 512],
                                    lhsT=ctxT8[
                                        :, 2 * ks:2 * ks + 2,
                                        m * 128:(m + 1) * 128,
                                    ],
                                    rhs=wo8[
                                        :, 2 * ks:2 * ks + 2,
                                        n * 512:(n + 1) * 512,
                                    ],
                                    start=False, stop=(ks == 3),
                                    perf_mode=PM.DoubleRow,
                                )
                        layernorm(ao, hbf[:, m, :], g1bc, b1lnbc, p_ln1)

                    def ht_half(n):
                        for b in range(KT):
                            tr = pp_tr7.tile([128, 512], BF16, tag="tr7")
                            for a in range(4):
                                nc.tensor.transpose(
                                    tr[:, a * 128:(a + 1) * 128],
                                    hbf[:, n * 4 + a, b * 128:(b + 1) * 128],
                                    identb,
                                )
                            if b < 6:
                                nc.vector.tensor_scalar_mul(
                                    hT8[:, b, n * 512:(n + 1) * 512], tr, 8.0
                                )
                            else:
                                nc.vector.tensor_scalar_mul(
                                    hTb[:, b - 6, n * 512:(n + 1) * 512],
                                    tr, 16.0,
                                )

                    def f1_half(j, n, dve_relu=False):
                        ps = pp_f1.tile([128, 512], F32, tag="f1h")
                        w8c, wtc = w1cs[j // 4]
                        jj = j % 4
                        for p in range(3):
                            nc.tensor.matmul(
                                ps,
                                lhsT=w8c[:, p, :, jj, :],
                                rhs=hT8[:, 2 * p:2 * p + 2,
                                        n * 512:(n + 1) * 512],
                                start=(p == 0), stop=False,
                                perf_mode=PM.DoubleRow,
                            )
                        for k in range(2):
                            nc.tensor.matmul(
                                ps,
                                lhsT=wtc[:, k, jj, :],
                                rhs=hTb[:, k, n * 512:(n + 1) * 512],
                                start=False, stop=(k == 1),
                            )
                        if dve_relu and not has_b1:
                            # LN1 window: ACT does the LN chain and DVE the
                            # stats there; run these fillers' ReLU on the
                            # idle GpSimd so the f1 PSUM slots recycle
                            # without delaying either chain engine
                            nc.vector.tensor_scalar(
                                out=ff1[:, j, n * 512:(n + 1) * 512],
                                in0=ps, scalar1=1.0 / 256.0, scalar2=0.0,
                                op0=OP.mult, op1=OP.max,
                            )
                            return
                        bias = b1sb[:, j:j + 1] if has_b1 else 0.0
                        nc.scalar.activation(
                            ff1[:, j, n * 512:(n + 1) * 512], ps,
                            AF.Relu, bias=bias, scale=1.0 / 256.0,
                        )

                    for m in range(MT):
                        if m + 2 < MT:
                            xt_dma(m + 2)
                        op_ln1(m)
                        if m == 3:
                            ht_half(0)
                        elif m >= 4:
                            f1_half(m - 4, 0, dve_relu=True)
                    for j in range(4, 16):
                        f1_half(j, 0, dve_relu=True)
                    ht_half(1)
                    w2_dma(0)
                    for jc in range(4):
                        for jj in range(4):
                            f1_half(jc * 4 + jj, 1)
                        w1_dma(4 + jc)
                        w2_dma(1 + jc)
                    for jc in range(4, 8):
                        if jc < 7:
                            w2_dma(jc + 1)
                        for jj in range(4):
                            j = jc * 4 + jj
                            f1_half(j, 0)
                            f1_half(j, 1)

                  # ---- FFN2 + residual + LN2 + out^T ----
                  with (
                    tc.tile_pool(name="p_ln2", bufs=2) as p_ln2,
                    tc.tile_pool(name="p_z2", bufs=2) as p_z2,
                    tc.tile_pool(name="p_out", bufs=4) as p_out,
                    tc.tile_pool(name="pp_f2", bufs=1, space="PSUM") as pp_f2,
                    tc.tile_pool(name="pp_t11", bufs=2, space="PSUM") as pp_t11,
                  ):
                    def ffn2_m(m):
                        ps = pp_f2.tile([128, C], F32, tag=f"f2_{m % 2}",
                                        name=f"f2ps_{m}")
                        for n in range(2):
                            nc.tensor.matmul(
                                ps[:, n * 512:(n + 1) * 512],
                                lhsT=identb,
                                rhs=hbf[:, m, n * 512:(n + 1) * 512],
                                start=True, stop=False,
                            )
                            for j in range(JT):
                                nc.tensor.matmul(
                                    ps[:, n * 512:(n + 1) * 512],
                                    lhsT=ff1[:, j, m * 128:(m + 1) * 128],
                                    rhs=w2sb[:, j, n * 512:(n + 1) * 512],
                                    start=False, stop=(j == JT - 1),
                                )
                        return ps

                    def ln2_m(m, ps):
                        if has_b2:
                            resid2 = p_ln2.tile([128, C], F32, tag="resid2")
                            nc.vector.tensor_add(resid2, ps, b2bc)
                            src = resid2
                        else:
                            src = ps
                        zt = p_z2.tile([128, C], BF16, tag="z2",
                                       name=f"z2_{m}")
                        layernorm(src, zt, g2bc, b2lnbc, p_ln2)
                        return zt

                    def outT(m, zt):
                        # bf16 transposes (final LN2 output; ~0.1% rounding)
                        for b in range(KT):
                            tr = pp_t11.tile([128, 128], BF16, tag="t11")
                            nc.tensor.transpose(
                                tr, zt[:, b * 128:(b + 1) * 128], identb,
                            )
                            osb = p_out.tile([128, 128], F32, tag="osb")
                            nc.vector.tensor_copy(osb, tr)
                            eng = nc.sync if b % 2 == 0 else nc.scalar
                            eng.dma_start(
                                out=out_d[
                                    b * 128:(b + 1) * 128,
                                    m * 128:(m + 1) * 128,
                                ],
                                in_=osb,
                            )

                    for m in range(MT):
                        ps = ffn2_m(m)
                        outT(m, ln2_m(m, ps))
    _split_multiwait(nc)
    return nc


_prog_cache = {}


def prepare(
    x, in_proj_w, out_proj_w, ln1_g, ln1_b, ln2_g, ln2_b, w1, b1, w2, b2
):
    """Returns (nc, in_maps): the built program plus per-core input maps."""
    x = np.asarray(x, dtype=np.float32)
    in_proj_w = np.asarray(in_proj_w, dtype=np.float32)
    out_proj_w = np.asarray(out_proj_w, dtype=np.float32)
    ln1_g = np.asarray(ln1_g, dtype=np.float32)
    ln1_b = np.asarray(ln1_b, dtype=np.float32)
    ln2_g = np.asarray(ln2_g, dtype=np.float32)
    ln2_b = np.asarray(ln2_b, dtype=np.float32)
    w1 = np.asarray(w1, dtype=np.float32)
    b1 = np.asarray(b1, dtype=np.float32)
    w2 = np.asarray(w2, dtype=np.float32)
    b2 = np.asarray(b2, dtype=np.float32)

    flags = (
        not np.all(ln1_g == 1.0),
        not np.all(ln1_b == 0.0),
        not np.all(ln2_g == 1.0),
        not np.all(ln2_b == 0.0),
        not np.all(b1 == 0.0),
        not np.all(b2 == 0.0),
    )
    if flags not in _prog_cache:
        _prog_cache[flags] = _build(flags)
    nc = _prog_cache[flags]

    peT, ident, mask01 = _host_constants()

    winT = np.ascontiguousarray(in_proj_w.T)             # [C, 3C] f32
    wqk = np.ascontiguousarray(np.transpose(
        (winT[:, :2 * C] * SW).reshape(KT, 128, 16, 128), (1, 2, 0, 3)
    )).astype(E4)                                        # [128, 16, KT, 128]
    wv = np.ascontiguousarray(np.transpose(
        (winT[:, 2 * C:] * SW).reshape(KT, 128, C), (1, 0, 2)
    )).astype(E4)                                        # [128, KT, C]
    wo = np.ascontiguousarray(np.transpose(
        (out_proj_w.T * SWO).reshape(KT, 128, C), (1, 0, 2)
    )).astype(E4)                                        # [128, KT, C]
    w1r = np.transpose(
        w1.T.reshape(KT, 128, JT, 128), (1, 0, 2, 3)
    )                                                    # [128, KT, JT, 128]
    w18 = np.ascontiguousarray(
        (w1r[:, 0:6] * 32.0).reshape(128, 3, 2, JT, 128)
    ).astype(E4)                                         # [128, 3, 2, JT, 128]
    w1t = np.ascontiguousarray(w1r[:, 6:8] * 16.0).astype(BF)
    w2b = np.ascontiguousarray(np.transpose(
        w2.T.reshape(JT, 128, C), (1, 0, 2)
    )).astype(BF)                                        # [128, JT, C]

    shared = {
        "wqk8": wqk,
        "wv8": wv,
        "wo8": wo,
        "w18": w18,
        "w1t": w1t,
        "w2b": w2b,
        "identf": ident.astype(np.float32),
        "identb": ident.astype(BF),
        "mask01": mask01,
    }
    if flags[0]:
        shared["g1"] = ln1_g
    if flags[1]:
        shared["b1ln"] = ln1_b
    if flags[2]:
        shared["g2"] = ln2_g
    if flags[3]:
        shared["b2ln"] = ln2_b
    if flags[4]:
        shared["b1t"] = np.ascontiguousarray(b1.reshape(JT, 128).T)
    if flags[5]:
        shared["b2"] = b2

    in_maps = []
    for i in range(N):
        m = dict(shared)
        xq = (x[i] + peT) * SX                            # [C, T]
        m["xq8"] = np.ascontiguousarray(
            np.transpose(xq.reshape(KT, 128, T), (1, 0, 2))
        ).astype(E4)                                      # [128, KT, T]
        # residual ships pre-scaled by SV*SWO (the attention-psum scale;
        # LN is scale-invariant) so it can ride the out-proj matmul group
        m["x_tc"] = np.ascontiguousarray(
            x[i].T * (SV * SWO)
        ).astype(BF)                                      # [T, C] bf16
        in_maps.append(m)
    return nc, in_maps


def kernel(**inputs):
    nc, in_maps = prepare(**inputs)
    res = run_bass_kernel_spmd(nc, in_maps, core_ids=list(range(N)))
    out = np.stack([res.results[i]["out"] for i in range(N)], axis=0)
    return out.astype(np.float32)



# revision 59
# speedup vs baseline: 1.1804x; 1.0036x over previous
"""Trainium2 Bass kernel for nn_MhaSelfAttenLayer (dense transformer layer).

Data-parallel over batch: each of the 8 NeuronCores runs the full layer on
one batch element. No collectives.

Precision plan (validated numerically; sim matches HW to ~1e-4):
- Attention path (QKV proj, V, out proj, attn*V) in fp8 e4m3 with DoubleRow
  matmuls; scores in bf16 (64-wide contraction gets no DoubleRow benefit);
  fp32 PSUM accumulation everywhere.
- FFN1 contracts 6 of 8 c-tiles in fp8 DoubleRow (x32/x8 scales) and the
  last 2 in bf16 (x16/x16, so both partials carry the same 256x PSUM
  scale, removed in the ReLU activation scale).  7/8 or 8/8 fp8 breaches
  the 2e-2 budget; 6/8 lands at 1.89e-2.  FFN2 stays bf16.
- Residual adds ride the PE: each out-proj / FFN2 PSUM accumulation group
  starts with identity.T @ residual (bf16), and the LayerNorms read stats
  straight from PSUM.  LN is scale-invariant, so the fp8 512x scale on the
  attention PSUM needs no unscale; x ships pre-scaled as bf16.
- Scale folding: qkT holds 256*q ; the 256^2 factor is removed inside the
  exp() activation scale (exact power of two), v/ctx/out-proj scales fold
  into existing copies, so fp8 costs no extra instructions.
"""

import math

import numpy as np
import ml_dtypes

import concourse.bass as bass
import concourse.tile as tile
from concourse import mybir
from concourse.bass_utils import run_bass_kernel_spmd
from concourse.vector_clock import ScopedClock, VectorClock

F32 = mybir.dt.float32
BF16 = mybir.dt.bfloat16
F8 = mybir.dt.float8e4
BF = ml_dtypes.bfloat16
E4 = ml_dtypes.float8_e4m3

N, T, C, H, HD, HID = 8, 1024, 1024, 16, 64, 4096
KT = C // 128           # 8 c-tiles
MT = T // 128           # 8 t-tiles
JT = HID // 128         # 32 hid-tiles
EPS = 1e-5
AF = mybir.ActivationFunctionType
OP = mybir.AluOpType
PM = mybir.MatmulPerfMode

SX = 8.0                # xq8 = SX*(x+pe)
SW = 32.0               # wqk8 = SW*w  -> qkT = 256*q
ESC = 1.0 / (256.0 * 256.0 * 8.0)   # exp arg unscale (2^-19), incl 1/sqrt(hd)
SE = 2.0                # ex = SE*exp(score)
SV = 16.0               # v8 = SV*v ; ctxT8 = SV*ctx
SWO = 32.0              # wo8 = SWO*wo
AOS = 1.0 / (SV * SWO)  # attn-out unscale

_patched = False


def _patch_drain():
    """This walrus build accepts at most 1 sem wait per instruction (2 for
    EventSemaphore). Tile's final drain packs every outstanding proc wait
    onto a single drain -> codegen error. Emit one drain per proc instead."""
    global _patched
    if _patched:
        return
    _patched = True

    def _split_drain_and_barrier(self, tick_clock, wait_clock):
        gclock = tick_clock.global_clock
        n = len(gclock)
        for proc in range(n):
            t = gclock[proc]
            if t <= 0:
                continue
            vc = VectorClock([0] * n)
            vc.require_at_least(proc, t)
            d = self.nc.sync.drain()
            wait_clock.add_sem_waits(d.ins, ScopedClock({None: vc}))
        self.nc.all_engine_barrier()
        popped = self.nc._tile_sem_poison_stack.pop()
        assert popped is self._sem_poison
        self.nc.clear_and_free_semaphores(list(self.sems.allocated().values()))
        self.nc.all_engine_barrier()

    tile.TileContext._drain_and_barrier = _split_drain_and_barrier


def _split_multiwait(nc):
    """This walrus build accepts at most one sem wait per instruction. Hoist
    excess waits onto freshly created same-engine nops placed immediately
    before the over-limit instruction (engine streams run in order, so the
    nop blocking first preserves the dependency)."""
    import bass_rust

    engmap = {
        mybir.EngineType.PE: nc.tensor,
        mybir.EngineType.DVE: nc.vector,
        mybir.EngineType.Activation: nc.scalar,
        mybir.EngineType.SP: nc.sync,
        mybir.EngineType.Pool: nc.gpsimd,
    }
    blocks = list(nc.main_func.blocks)
    records = []
    for bi, bb in enumerate(blocks):
        for ins in bb.instructions:
            si = ins.sync_info
            if si is None or not si.on_wait:
                continue
            waits = list(si.on_wait)
            limit = 2 if type(ins).__name__ == "InstEventSemaphore" else 1
            if len(waits) > limit:
                records.append((ins.name, ins, waits[:-limit]))
                si.on_wait = waits[-limit:]
    if not records:
        return
    carriers = {}
    nop_names = set()
    for name, ins, excess in records:
        lst = []
        for w in excess:
            nb = engmap[ins.engine].nop()
            nb.ins.sync_info = bass_rust.SyncInfo(on_wait=[w], on_update=[])
            nop_names.add(nb.ins.name)
            lst.append(nb.ins)
        carriers[name] = lst
    for bb in blocks:
        il = list(bb.instructions)
        out = []
        changed = False
        for ins in il:
            if ins.name in nop_names:
                changed = True
                continue
            if ins.name in carriers:
                out.extend(carriers[ins.name])
                changed = True
            out.append(ins)
        if changed:
            bb.instructions = out


def _host_constants():
    pos = np.arange(T, dtype=np.float32)[:, None]
    div = np.exp(
        np.arange(0, C, 2, dtype=np.float32) * (-math.log(10000.0) / C)
    )
    ang = pos * div
    pe = np.stack([np.sin(ang), np.cos(ang)], axis=-1).reshape(T, C)
    peT = np.ascontiguousarray(pe.T)                    # [C, T]

    ident = np.eye(128, dtype=np.float32)
    kk = np.arange(128)
    # mask01[k, q] = 0 where q < k (future key within diagonal block)
    mask01 = np.where(kk[None, :] < kk[:, None], 0.0, 1.0).astype(E4)
    return peT, ident, mask01


def _build(flags):
    """flags = (g1, b1ln, g2, b2ln, b1, b2) booleans for non-trivial params."""
    has_g1, has_b1ln, has_g2, has_b2ln, has_b1, has_b2 = flags
    _patch_drain()
    nc = bass.Bass(trn_type="TRN2")

    # ---- DRAM I/O ----
    xq_d = nc.dram_tensor("xq8", [128, KT, T], F8, kind="ExternalInput")
    x_tc = nc.dram_tensor("x_tc", [T, C], BF16, kind="ExternalInput")
    wqk_d = nc.dram_tensor("wqk8", [128, 16, KT, 128], F8, kind="ExternalInput")
    wv_d = nc.dram_tensor("wv8", [128, KT, C], F8, kind="ExternalInput")
    wo_d = nc.dram_tensor("wo8", [128, KT, C], F8, kind="ExternalInput")
    w18_d = nc.dram_tensor("w18", [128, 3, 2, JT, 128], F8,
                           kind="ExternalInput")
    w1t_d = nc.dram_tensor("w1t", [128, 2, JT, 128], BF16,
                           kind="ExternalInput")
    w2_d = nc.dram_tensor("w2b", [128, JT, C], BF16, kind="ExternalInput")
    idb_d = nc.dram_tensor("identb", [128, 128], BF16, kind="ExternalInput")
    mk_d = nc.dram_tensor("mask01", [128, 128], F8, kind="ExternalInput")
    if has_g1:
        g1_d = nc.dram_tensor("g1", [C], F32, kind="ExternalInput")
    if has_b1ln:
        b1ln_d = nc.dram_tensor("b1ln", [C], F32, kind="ExternalInput")
    if has_g2:
        g2_d = nc.dram_tensor("g2", [C], F32, kind="ExternalInput")
    if has_b2ln:
        b2ln_d = nc.dram_tensor("b2ln", [C], F32, kind="ExternalInput")
    if has_b1:
        b1_d = nc.dram_tensor("b1t", [128, JT], F32, kind="ExternalInput")
    if has_b2:
        b2_d = nc.dram_tensor("b2", [C], F32, kind="ExternalInput")
    out_d = nc.dram_tensor("out", [C, T], F32, kind="ExternalOutput")

    def bcast_ap(dram_1d, n):
        return bass.AP(tensor=dram_1d.tensor, offset=0, ap=[[0, 128], [1, n]])

    with tile.TileContext(nc) as tc:
        with (
            tc.tile_pool(name="consts", bufs=1) as consts,
            tc.tile_pool(name="smalls", bufs=12) as smalls,
            tc.tile_pool(name="p_hbf", bufs=1) as p_hbf,
            tc.tile_pool(name="p_hT", bufs=1) as p_hT,
        ):
            # ---- constants (ACT-ring DMAs; SP ring is kept clear for the
            # latency-critical xq8/wqk loads) ----
            zbias = consts.tile([128, 1], F32)
            nc.vector.memset(zbias, 0.0)
            nc.const_aps.aps[(F32, 0.0)] = zbias
            epsb = consts.tile([128, 1], F32)
            nc.vector.memset(epsb, EPS)
            lnb = consts.tile([128, 1], F32)
            nc.vector.memset(lnb, float(math.log(SE)))
            warm = consts.tile([128, 64], BF16)
            nc.vector.memset(warm, 0.25)
            tblw = smalls.tile([128, 1], F32, tag="tblw")
            nc.scalar.activation(tblw, epsb, AF.Exp, bias=0.0, scale=1.0)
            nc.scalar.activation(tblw, epsb, AF.Sqrt, bias=0.0, scale=1.0)
            mask01 = consts.tile([128, 128], F8)
            nc.gpsimd.dma_start(out=mask01, in_=mk_d[:, :])
            identb = consts.tile([128, 128], BF16)
            nc.gpsimd.dma_start(out=identb, in_=idb_d[:, :])
            g1bc = b1lnbc = g2bc = b2lnbc = b1sb = b2bc = None
            if has_g1:
                g1bc = consts.tile([128, C], F32)
                nc.scalar.dma_start(out=g1bc, in_=bcast_ap(g1_d, C))
            if has_b1ln:
                b1lnbc = consts.tile([128, C], F32)
                nc.scalar.dma_start(out=b1lnbc, in_=bcast_ap(b1ln_d, C))
            if has_g2:
                g2bc = consts.tile([128, C], F32)
                nc.scalar.dma_start(out=g2bc, in_=bcast_ap(g2_d, C))
            if has_b2ln:
                b2lnbc = consts.tile([128, C], F32)
                nc.scalar.dma_start(out=b2lnbc, in_=bcast_ap(b2ln_d, C))
            if has_b1:
                b1sb = consts.tile([128, JT], F32)
                nc.scalar.dma_start(out=b1sb, in_=b1_d[:, :])
            if has_b2:
                b2bc = consts.tile([128, C], F32)
                nc.scalar.dma_start(out=b2bc, in_=bcast_ap(b2_d, C))

            # ---- PE warm-up: ~6us of tiny matmuls during the input-DMA
            # wait so the HAM clock-gate reaches K=8/8 before real work ----
            with tc.tile_pool(name="pp_warm", bufs=1, space="PSUM") as pp_w:
                wps = pp_w.tile([64, 64], F32)
                for _ in range(84):
                    nc.tensor.matmul(wps, lhsT=warm[:, 0:64],
                                     rhs=warm[:, 0:64], start=True, stop=True)

            hbf = p_hbf.tile([128, MT, C], BF16)
            hT8 = p_hT.tile([128, 6, T], F8)
            hTb = p_hT.tile([128, 2, T], BF16)

            def layernorm(resid, out_tile, gbc, bbc, zpool):
                stats = smalls.tile([128, 2, 6], F32, tag="stats")
                nc.vector.bn_stats(out=stats[:, 0, :], in_=resid[:, 0:512])
                nc.vector.bn_stats(out=stats[:, 1, :], in_=resid[:, 512:1024])
                mv = smalls.tile([128, 2], F32, tag="mv")
                nc.vector.bn_aggr(out=mv, in_=stats)
                std = smalls.tile([128, 1], F32, tag="std")
                nc.scalar.activation(std, mv[:, 1:2], AF.Sqrt, bias=epsb)
                istd = smalls.tile([128, 1], F32, tag="istd")
                nc.vector.reciprocal(istd, std)
                nbias = smalls.tile([128, 1], F32, tag="nbias")
                nc.vector.tensor_scalar(
                    out=nbias, in0=mv[:, 0:1], scalar1=istd, scalar2=-1.0,
                    op0=OP.mult, op1=OP.mult,
                )
                if gbc is None and bbc is None:
                    # two halves: downstream per-128-col transposes unblock
                    # after half 0 (subtile deps), hiding the chain latency
                    nc.scalar.activation(
                        out_tile[:, 0:512], resid[:, 0:512],
                        AF.Identity, bias=nbias, scale=istd,
                    )
                    nc.scalar.activation(
                        out_tile[:, 512:1024], resid[:, 512:1024],
                        AF.Identity, bias=nbias, scale=istd,
                    )
                else:
                    z = zpool.tile([128, C], F32, tag="zln")
                    nc.vector.tensor_scalar(
                        out=z, in0=resid, scalar1=istd, scalar2=nbias,
                        op0=OP.mult, op1=OP.add,
                    )
                    if gbc is not None:
                        nc.vector.tensor_mul(z, z, gbc)
                    if bbc is not None:
                        nc.vector.tensor_add(z, z, bbc)
                    nc.vector.tensor_copy(out_tile, z)

            # ================= front: QKV + attention =================
            with (
                tc.tile_pool(name="p_ctxT", bufs=1) as p_ctxT,
                tc.tile_pool(name="p_ln1", bufs=3) as p_ln1,
                tc.tile_pool(name="p_w1", bufs=4) as p_w1,
            ):
                ctxT8 = p_ctxT.tile([128, KT, T], F8)
                wo8 = p_ctxT.tile([128, KT, C], F8)
                # xt / w1 pools live OUTSIDE the attention pools so their
                # DMAs have no address-reuse deps and land during attention
                xts = {}

                def xt_dma(m):
                    xt = p_ln1.tile([128, C], BF16, tag="xt",
                                    name=f"xt_{m}")
                    nc.sync.dma_start(
                        out=xt, in_=x_tc[m * 128:(m + 1) * 128, :]
                    )
                    xts[m] = xt

                w1cs = {}

                def w1_dma(jc):
                    w8c = p_w1.tile([128, 3, 2, 4, 128], F8, tag="w8c",
                                    name=f"w8c_{jc}")
                    nc.gpsimd.dma_start(
                        out=w8c, in_=w18_d[:, :, :, jc * 4:(jc + 1) * 4, :]
                    )
                    wtc = p_w1.tile([128, 2, 4, 128], BF16, tag="wtc",
                                    name=f"wtc_{jc}")
                    nc.gpsimd.dma_start(
                        out=wtc, in_=w1t_d[:, :, jc * 4:(jc + 1) * 4, :]
                    )
                    w1cs[jc] = (w8c, wtc)

                with (
                    tc.tile_pool(name="p_att", bufs=1) as p_att,
                    tc.tile_pool(name="p_ex", bufs=4) as p_ex,
                    tc.tile_pool(name="pp_big", bufs=7, space="PSUM") as pp_big,
                    tc.tile_pool(name="pp_ctx", bufs=1, space="PSUM") as pp_ctx,
                ):
                    qkT = p_att.tile([128, 16, T], BF16)
                    vsb = p_att.tile([128, MT, H * (HD + 1)], F8)
                    v4 = vsb.rearrange("p m (h e) -> p m h e", h=H)
                    ctxf = p_att.tile([128, MT, C], BF16)
                    xq8 = p_att.tile([128, KT, T], F8)
                    wqk8 = p_att.tile([128, 16, KT, 128], F8)
                    wv8 = p_att.tile([128, KT, C], F8)

                    nc.sync.dma_start(out=xq8[:, 0:2, :],
                                      in_=xq_d[:, 0:2, :])
                    nc.scalar.dma_start(out=xq8[:, 2:4, :],
                                      in_=xq_d[:, 2:4, :])
                    nc.gpsimd.dma_start(out=xq8[:, 4:6, :],
                                        in_=xq_d[:, 4:6, :])
                    nc.scalar.dma_start(out=xq8[:, 6:8, :],
                                      in_=xq_d[:, 6:8, :])
                    nc.sync.dma_start(out=wqk8[:, 0, :, :],
                                      in_=wqk_d[:, 0, :, :])
                    nc.sync.dma_start(out=wqk8[:, 8, :, :],
                                      in_=wqk_d[:, 8, :, :])
                    for lo, hi in ((1, 3), (9, 11), (3, 6), (11, 14),
                                   (6, 8), (14, 16)):
                        nc.sync.dma_start(out=wqk8[:, lo:hi, :, :],
                                          in_=wqk_d[:, lo:hi, :, :])
                    nc.scalar.dma_start(out=wv8, in_=wv_d[:, :, :])
                    nc.gpsimd.dma_start(out=wo8, in_=wo_d[:, :, :])
                    w1_dma(0)
                    w1_dma(1)
                    w1_dma(2)
                    w1_dma(3)
                    xt_dma(0)
                    xt_dma(1)
                    nc.vector.memset(v4[:, :, :, HD:HD + 1], 1.0)

                    def qkv_unit(m, n):
                        def u():
                            mm = pp_big.tile([128, 512], F32, tag="big")
                            for ks in range(4):
                                nc.tensor.matmul(
                                    mm,
                                    lhsT=wqk8[:, m, 2 * ks:2 * ks + 2, :],
                                    rhs=xq8[
                                        :, 2 * ks:2 * ks + 2,
                                        n * 512:(n + 1) * 512,
                                    ],
                                    start=(ks == 0), stop=(ks == 3),
                                    perf_mode=PM.DoubleRow,
                                )
                            nc.vector.tensor_copy(
                                qkT[:, m, n * 512:(n + 1) * 512], mm
                            )
                        return u

                    def vproj_unit(m, n):
                        def u():
                            mm = pp_big.tile([128, 512], F32, tag="big")
                            for ks in range(4):
                                nc.tensor.matmul(
                                    mm,
                                    lhsT=xq8[
                                        :, 2 * ks:2 * ks + 2,
                                        m * 128:(m + 1) * 128,
                                    ],
                                    rhs=wv8[
                                        :, 2 * ks:2 * ks + 2,
                                        n * 512:(n + 1) * 512,
                                    ],
                                    start=(ks == 0), stop=(ks == 3),
                                    perf_mode=PM.DoubleRow,
                                )
                            nc.vector.tensor_scalar_mul(
                                v4[:, m, n * 8:(n + 1) * 8, 0:HD],
                                mm.rearrange("p (h e) -> p h e", h=8),
                                1.0 / SV,
                            )
                        return u

                    exmap = {}

                    def sc_pair_units(a):
                        """Score+exp units for head pair (2a, 2a+1).  The even
                        head's kt/qt live on partitions 0-63, the odd head's on
                        64-127, so the two back-to-back matmuls land on disjoint
                        PE row groups (tile_position auto-derives) and execute
                        concurrently — 2x score throughput."""
                        he, ho = 2 * a, 2 * a + 1
                        exe = p_ex.tile([128, MT, T], F8, tag="ex",
                                        name=f"ex_{he}")
                        exo = p_ex.tile([128, MT, T], F8, tag="ex",
                                        name=f"ex_{ho}")
                        exmap[he], exmap[ho] = exe, exo
                        units = []
                        for j in range(MT):
                            qspan = T - j * 128
                            for lo in range(0, qspan, 512):
                                hi = min(lo + 512, qspan)

                                def u(j=j, lo=lo, hi=hi):
                                    qlo, qhi = j * 128 + lo, j * 128 + hi
                                    ste = pp_big.tile(
                                        [128, 512], F32, tag="big"
                                    )
                                    sto = pp_big.tile(
                                        [128, 512], F32, tag="big"
                                    )
                                    nc.tensor.matmul(
                                        ste[:, 0:hi - lo],
                                        lhsT=qkT[0:64, 8 + a,
                                                 j * 128:(j + 1) * 128],
                                        rhs=qkT[0:64, a, qlo:qhi],
                                        start=True, stop=True,
                                    )
                                    nc.tensor.matmul(
                                        sto[:, 0:hi - lo],
                                        lhsT=qkT[64:128, 8 + a,
                                                 j * 128:(j + 1) * 128],
                                        rhs=qkT[64:128, a, qlo:qhi],
                                        start=True, stop=True,
                                    )
                                    for exh, st in ((exe, ste), (exo, sto)):
                                        nc.scalar.activation(
                                            exh[:, j, qlo:qhi],
                                            st[:, 0:hi - lo],
                                            AF.Exp, bias=lnb, scale=ESC,
                                        )
                                        if lo == 0:
                                            nc.gpsimd.tensor_mul(
                                                exh[:, j,
                                                    j * 128:(j + 1) * 128],
                                                exh[:, j,
                                                    j * 128:(j + 1) * 128],
                                                mask01,
                                            )
                                units.append(u)
                        return units

                    def ctx_unit(h, i2):
                        exh = exmap[h]

                        def u():
                            # normal-mode fp8: 128-col weight loads keep the
                            # compiler's fast-weight-load (DoubleRow at free
                            # dim 65 is LDWEIGHTS-bound and congests the
                            # weight port for qkv/score loads)
                            cps = pp_ctx.tile([128, 4, HD + 1], F32, tag="cps")
                            for ii in range(4):
                                i = i2 * 4 + ii
                                for j in range(i + 1):
                                    nc.tensor.matmul(
                                        cps[:, ii, :],
                                        lhsT=exh[:, j, i * 128:(i + 1) * 128],
                                        rhs=v4[:, j, h, :],
                                        start=(j == 0), stop=(j == i),
                                    )
                            rden = smalls.tile([128, 4], F32, tag="rden")
                            nc.vector.reciprocal(
                                rden,
                                cps.rearrange("p i e -> p (i e)")[:, HD::HD + 1],
                            )
                            rb = bass.AP(
                                tensor=rden.tensor, offset=rden.offset,
                                ap=[rden.ap[0], [rden.ap[1][0], 4], [0, HD]],
                            )
                            nc.vector.tensor_mul(
                                ctxf[:, i2 * 4:(i2 + 1) * 4,
                                     h * HD:(h + 1) * HD],
                                cps[:, :, 0:HD], rb,
                            )
                        return u

                    def p4_unit(b, n):
                        def u():
                            tr = pp_big.tile([128, 512], BF16, tag="big")
                            for a in range(4):
                                nc.tensor.transpose(
                                    tr[:, a * 128:(a + 1) * 128],
                                    ctxf[:, n * 4 + a, b * 128:(b + 1) * 128],
                                    identb,
                                )
                            nc.vector.tensor_copy(
                                ctxT8[:, b, n * 512:(n + 1) * 512], tr
                            )
                        return u

                    # --- software-pipelined emission: paired score chunks
                    # (2x concurrent via row tiling) are spread uniformly
                    # between other PE work so the exp stream never starves;
                    # qkT pairs are produced one slot ahead. ---
                    for u in (qkv_unit(0, 0), qkv_unit(0, 1),
                              qkv_unit(8, 0), qkv_unit(8, 1)):
                        u()
                    for a in range(8):
                        others = []
                        if a + 1 < 8:
                            others += [qkv_unit(a + 1, 0), qkv_unit(a + 1, 1),
                                       qkv_unit(9 + a, 0), qkv_unit(9 + a, 1)]
                        if a == 0:
                            others += [vproj_unit(m, n)
                                       for m in range(8) for n in range(2)]
                        if a >= 1:
                            hp = 2 * (a - 1)
                            others += [ctx_unit(hp, 0), ctx_unit(hp, 1),
                                       ctx_unit(hp + 1, 0), ctx_unit(hp + 1, 1)]
                        if a >= 2:
                            others += [p4_unit(a - 2, 0), p4_unit(a - 2, 1)]
                        sts = sc_pair_units(a)
                        ns = len(sts)
                        no = len(others)
                        for idx, su in enumerate(sts):
                            su()
                            for u in others[
                                (idx * no) // ns:((idx + 1) * no) // ns
                            ]:
                                u()
                    for u in (ctx_unit(H - 2, 0), ctx_unit(H - 2, 1),
                              ctx_unit(H - 1, 0), ctx_unit(H - 1, 1),
                              p4_unit(6, 0), p4_unit(6, 1),
                              p4_unit(7, 0), p4_unit(7, 1)):
                        u()

                # ---- out-proj + residual + LN1 + h^T + FFN1 overlap ----
                # Residual rides the PE: the out-proj PSUM group starts with
                # identb.T @ (512*x)  (bf16), then accumulates the fp8 DR
                # out-proj matmuls.  LN is scale-invariant, so the 512x
                # scale needs no unscale; bn_stats reads the PSUM directly.
                with (
                    tc.tile_pool(name="p_ff1", bufs=1) as p_ff1,
                    tc.tile_pool(name="p_w2r", bufs=1) as p_w2r,
                ):
                  ff1 = p_ff1.tile([128, JT, T], BF16)
                  w2sb = p_w2r.tile([128, JT, C], BF16)

                  def w2_dma(jc):
                      nc.gpsimd.dma_start(
                          out=w2sb[:, jc * 4:(jc + 1) * 4, :],
                          in_=w2_d[:, jc * 4:(jc + 1) * 4, :],
                      )

                  with (
                    tc.tile_pool(name="pp_ao", bufs=2, space="PSUM") as pp_ao,
                    tc.tile_pool(name="pp_f1", bufs=3, space="PSUM") as pp_f1,
                    tc.tile_pool(name="pp_tr7", bufs=1, space="PSUM") as pp_tr7,
                  ):
                    def op_ln1(m):
                        ao = pp_ao.tile([128, C], F32, tag="ao")
                        for n in range(2):
                            nc.tensor.matmul(
                                ao[:, n * 512:(n + 1) * 512],
                                lhsT=identb,
                                rhs=xts[m][:, n * 512:(n + 1) * 512],
                                start=True, stop=False,
                            )
                            for ks in range(4):
                                nc.tensor.matmul(
                                    ao[:, n * 512:(n + 1) * 512],
                                    lhsT=ctxT8[
                                        :, 2 * ks:2 * ks + 2,
                                        m * 128:(m + 1) * 128,
                                    ],
                                    rhs=wo8[
                                        :, 2 * ks:2 * ks + 2,
                                        n * 512:(n + 1) * 512,
                                    ],
                                    start=False, stop=(ks == 3),
                                    perf_mode=PM.DoubleRow,
                                )
                        layernorm(ao, hbf[:, m, :], g1bc, b1lnbc, p_ln1)

                    def ht_half(n):
                        for b in range(KT):
                            tr = pp_tr7.tile([128, 512], BF16, tag="tr7")
                            for a in range(4):
                                nc.tensor.transpose(
                                    tr[:, a * 128:(a + 1) * 128],
                                    hbf[:, n * 4 + a, b * 128:(b + 1) * 128],
                                    identb,
                                )
                            if b < 6:
                                nc.vector.tensor_scalar_mul(
                                    hT8[:, b, n * 512:(n + 1) * 512], tr, 8.0
                                )
                            else:
                                nc.vector.tensor_scalar_mul(
                                    hTb[:, b - 6, n * 512:(n + 1) * 512],
                                    tr, 16.0,
                                )

                    def f1_half(j, n, dve_relu=False):
                        ps = pp_f1.tile([128, 512], F32, tag="f1h")
                        w8c, wtc = w1cs[j // 4]
                        jj = j % 4
                        for p in range(3):
                            nc.tensor.matmul(
                                ps,
                                lhsT=w8c[:, p, :, jj, :],
                                rhs=hT8[:, 2 * p:2 * p + 2,
                                        n * 512:(n + 1) * 512],
                                start=(p == 0), stop=False,
                                perf_mode=PM.DoubleRow,
                            )
                        for k in range(2):
                            nc.tensor.matmul(
                                ps,
                                lhsT=wtc[:, k, jj, :],
                                rhs=hTb[:, k, n * 512:(n + 1) * 512],
                                start=False, stop=(k == 1),
                            )
                        if dve_relu and not has_b1:
                            # LN1 window: ACT does the LN chain and DVE the
                            # stats there; run these fillers' ReLU on the
                            # idle GpSimd so the f1 PSUM slots recycle
                            # without delaying either chain engine
                            nc.vector.tensor_scalar(
                                out=ff1[:, j, n * 512:(n + 1) * 512],
                                in0=ps, scalar1=1.0 / 256.0, scalar2=0.0,
                                op0=OP.mult, op1=OP.max,
                            )
                            return
                        bias = b1sb[:, j:j + 1] if has_b1 else 0.0
                        nc.scalar.activation(
                            ff1[:, j, n * 512:(n + 1) * 512], ps,
                            AF.Relu, bias=bias, scale=1.0 / 256.0,
                        )

                    for m in range(MT):
                        if m + 2 < MT:
                            xt_dma(m + 2)
                        op_ln1(m)
                        if m == 3:
                            ht_half(0)
                        elif m >= 4:
                            f1_half(m - 4, 0, dve_relu=True)
                    for j in range(4, 16):
                        f1_half(j, 0, dve_relu=True)
                    ht_half(1)
                    w2_dma(0)
                    for jc in range(4):
                        for jj in range(4):
                            f1_half(jc * 4 + jj, 1)
                        w1_dma(4 + jc)
                        w2_dma(1 + jc)
                    for jc in range(4, 8):
                        if jc < 7:
                            w2_dma(jc + 1)
                        for jj in range(4):
                            j = jc * 4 + jj
                            f1_half(j, 0)
                            f1_half(j, 1)

                  # ---- FFN2 + residual + LN2 + out^T ----
                  with (
                    tc.tile_pool(name="p_ln2", bufs=2) as p_ln2,
                    tc.tile_pool(name="p_z2", bufs=2) as p_z2,
                    tc.tile_pool(name="p_out", bufs=4) as p_out,
                    tc.tile_pool(name="pp_f2", bufs=1, space="PSUM") as pp_f2,
                    tc.tile_pool(name="pp_t11", bufs=2, space="PSUM") as pp_t11,
                  ):
                    def ffn2_m(m):
                        ps = pp_f2.tile([128, C], F32, tag=f"f2_{m % 2}",
                                        name=f"f2ps_{m}")
                        for n in range(2):
                            nc.tensor.matmul(
                                ps[:, n * 512:(n + 1) * 512],
                                lhsT=identb,
                                rhs=hbf[:, m, n * 512:(n + 1) * 512],
                                start=True, stop=False,
                            )
                            for j in range(JT):
                                nc.tensor.matmul(
                                    ps[:, n * 512:(n + 1) * 512],
                                    lhsT=ff1[:, j, m * 128:(m + 1) * 128],
                                    rhs=w2sb[:, j, n * 512:(n + 1) * 512],
                                    start=False, stop=(j == JT - 1),
                                )
                        return ps

                    def ln2_m(m, ps):
                        if has_b2:
                            resid2 = p_ln2.tile([128, C], F32, tag="resid2")
                            nc.vector.tensor_add(resid2, ps, b2bc)
                            src = resid2
                        else:
                            src = ps
                        zt = p_z2.tile([128, C], BF16, tag="z2",
                                       name=f"z2_{m}")
                        layernorm(src, zt, g2bc, b2lnbc, p_ln2)
                        return zt

                    def outT(m, zt):
                        # bf16 transposes (final LN2 output; ~0.1% rounding)
                        for b in range(KT):
                            tr = pp_t11.tile([128, 128], BF16, tag="t11")
                            nc.tensor.transpose(
                                tr, zt[:, b * 128:(b + 1) * 128], identb,
                            )
                            osb = p_out.tile([128, 128], F32, tag="osb")
                            nc.vector.tensor_copy(osb, tr)
                            eng = nc.sync if b % 2 == 0 else nc.scalar
                            eng.dma_start(
                                out=out_d[
                                    b * 128:(b + 1) * 128,
                                    m * 128:(m + 1) * 128,
                                ],
                                in_=osb,
                            )

                    for m in range(MT):
                        ps = ffn2_m(m)
                        outT(m, ln2_m(m, ps))
    _split_multiwait(nc)
    return nc


_prog_cache = {}


def prepare(
    x, in_proj_w, out_proj_w, ln1_g, ln1_b, ln2_g, ln2_b, w1, b1, w2, b2
):
    """Returns (nc, in_maps): the built program plus per-core input maps."""
    x = np.asarray(x, dtype=np.float32)
    in_proj_w = np.asarray(in_proj_w, dtype=np.float32)
    out_proj_w = np.asarray(out_proj_w, dtype=np.float32)
    ln1_g = np.asarray(ln1_g, dtype=np.float32)
    ln1_b = np.asarray(ln1_b, dtype=np.float32)
    ln2_g = np.asarray(ln2_g, dtype=np.float32)
    ln2_b = np.asarray(ln2_b, dtype=np.float32)
    w1 = np.asarray(w1, dtype=np.float32)
    b1 = np.asarray(b1, dtype=np.float32)
    w2 = np.asarray(w2, dtype=np.float32)
    b2 = np.asarray(b2, dtype=np.float32)

    flags = (
        not np.all(ln1_g == 1.0),
        not np.all(ln1_b == 0.0),
        not np.all(ln2_g == 1.0),
        not np.all(ln2_b == 0.0),
        not np.all(b1 == 0.0),
        not np.all(b2 == 0.0),
    )
    if flags not in _prog_cache:
        _prog_cache[flags] = _build(flags)
    nc = _prog_cache[flags]

    peT, ident, mask01 = _host_constants()

    winT = np.ascontiguousarray(in_proj_w.T)             # [C, 3C] f32
    wqk = np.ascontiguousarray(np.transpose(
        (winT[:, :2 * C] * SW).reshape(KT, 128, 16, 128), (1, 2, 0, 3)
    )).astype(E4)                                        # [128, 16, KT, 128]
    wv = np.ascontiguousarray(np.transpose(
        (winT[:, 2 * C:] * SW).reshape(KT, 128, C), (1, 0, 2)
    )).astype(E4)                                        # [128, KT, C]
    wo = np.ascontiguousarray(np.transpose(
        (out_proj_w.T * SWO).reshape(KT, 128, C), (1, 0, 2)
    )).astype(E4)                                        # [128, KT, C]
    w1r = np.transpose(
        w1.T.reshape(KT, 128, JT, 128), (1, 0, 2, 3)
    )                                                    # [128, KT, JT, 128]
    w18 = np.ascontiguousarray(
        (w1r[:, 0:6] * 32.0).reshape(128, 3, 2, JT, 128)
    ).astype(E4)                                         # [128, 3, 2, JT, 128]
    w1t = np.ascontiguousarray(w1r[:, 6:8] * 16.0).astype(BF)
    w2b = np.ascontiguousarray(np.transpose(
        w2.T.reshape(JT, 128, C), (1, 0, 2)
    )).astype(BF)                                        # [128, JT, C]

    shared = {
        "wqk8": wqk,
        "wv8": wv,
        "wo8": wo,
        "w18": w18,
        "w1t": w1t,
        "w2b": w2b,
        "identf": ident.astype(np.float32),
        "identb": ident.astype(BF),
        "mask01": mask01,
    }
    if flags[0]:
        shared["g1"] = ln1_g
    if flags[1]:
        shared["b1ln"] = ln1_b
    if flags[2]:
        shared["g2"] = ln2_g
    if flags[3]:
        shared["b2ln"] = ln2_b
    if flags[4]:
        shared["b1t"] = np.ascontiguousarray(b1.reshape(JT, 128).T)
    if flags[5]:
        shared["b2"] = b2

    in_maps = []
    for i in range(N):
        m = dict(shared)
        xq = (x[i] + peT) * SX                            # [C, T]
        m["xq8"] = np.ascontiguousarray(
            np.transpose(xq.reshape(KT, 128, T), (1, 0, 2))
        ).astype(E4)                                      # [128, KT, T]
        # residual ships pre-scaled by SV*SWO (the attention-psum scale;
        # LN is scale-invariant) so it can ride the out-proj matmul group
        m["x_tc"] = np.ascontiguousarray(
            x[i].T * (SV * SWO)
        ).astype(BF)                                      # [T, C] bf16
        in_maps.append(m)
    return nc, in_maps


def kernel(**inputs):
    nc, in_maps = prepare(**inputs)
    res = run_bass_kernel_spmd(nc, in_maps, core_ids=list(range(N)))
    out = np.stack([res.results[i]["out"] for i in range(N)], axis=0)
    return out.astype(np.float32)



# revision 60
# speedup vs baseline: 1.2204x; 1.0339x over previous
"""Trainium2 Bass kernel for nn_MhaSelfAttenLayer (dense transformer layer).

Data-parallel over batch: each of the 8 NeuronCores runs the full layer on
one batch element. No collectives.

Precision plan (validated numerically; sim matches HW to ~1e-4):
- Attention path (QKV proj, V, out proj, attn*V) in fp8 e4m3 with DoubleRow
  matmuls; scores in bf16 (64-wide contraction gets no DoubleRow benefit);
  fp32 PSUM accumulation everywhere.
- FFN1 contracts 6 of 8 c-tiles in fp8 DoubleRow (x32/x8 scales) and the
  last 2 in bf16 (x16/x16, so both partials carry the same 256x PSUM
  scale, removed in the ReLU activation scale).  7/8 or 8/8 fp8 breaches
  the 2e-2 budget; 6/8 lands at 1.89e-2.  FFN2 stays bf16.
- Residual adds ride the PE: each out-proj / FFN2 PSUM accumulation group
  starts with identity.T @ residual (bf16), and the LayerNorms read stats
  straight from PSUM.  LN is scale-invariant, so the fp8 512x scale on the
  attention PSUM needs no unscale; x ships pre-scaled as bf16.
- Scale folding: qkT holds 256*q ; the 256^2 factor is removed inside the
  exp() activation scale (exact power of two), v/ctx/out-proj scales fold
  into existing copies, so fp8 costs no extra instructions.
"""

import math

import numpy as np
import ml_dtypes

import concourse.bass as bass
import concourse.tile as tile
from concourse import mybir
from concourse.bass_utils import run_bass_kernel_spmd
from concourse.vector_clock import ScopedClock, VectorClock

F32 = mybir.dt.float32
BF16 = mybir.dt.bfloat16
F8 = mybir.dt.float8e4
BF = ml_dtypes.bfloat16
E4 = ml_dtypes.float8_e4m3

N, T, C, H, HD, HID = 8, 1024, 1024, 16, 64, 4096
KT = C // 128           # 8 c-tiles
MT = T // 128           # 8 t-tiles
JT = HID // 128         # 32 hid-tiles
EPS = 1e-5
AF = mybir.ActivationFunctionType
OP = mybir.AluOpType
PM = mybir.MatmulPerfMode

SX = 8.0                # xq8 = SX*(x+pe)
SW = 32.0               # wqk8 = SW*w  -> qkT = 256*q
ESC = 1.0 / (256.0 * 256.0 * 8.0)   # exp arg unscale (2^-19), incl 1/sqrt(hd)
SE = 2.0                # ex = SE*exp(score)
SV = 16.0               # v8 = SV*v ; ctxT8 = SV*ctx
SWO = 32.0              # wo8 = SWO*wo
AOS = 1.0 / (SV * SWO)  # attn-out unscale

_patched = False


def _patch_drain():
    """This walrus build accepts at most 1 sem wait per instruction (2 for
    EventSemaphore). Tile's final drain packs every outstanding proc wait
    onto a single drain -> codegen error. Emit one drain per proc instead."""
    global _patched
    if _patched:
        return
    _patched = True

    def _split_drain_and_barrier(self, tick_clock, wait_clock):
        gclock = tick_clock.global_clock
        n = len(gclock)
        for proc in range(n):
            t = gclock[proc]
            if t <= 0:
                continue
            vc = VectorClock([0] * n)
            vc.require_at_least(proc, t)
            d = self.nc.sync.drain()
            wait_clock.add_sem_waits(d.ins, ScopedClock({None: vc}))
        self.nc.all_engine_barrier()
        popped = self.nc._tile_sem_poison_stack.pop()
        assert popped is self._sem_poison
        self.nc.clear_and_free_semaphores(list(self.sems.allocated().values()))
        self.nc.all_engine_barrier()

    tile.TileContext._drain_and_barrier = _split_drain_and_barrier


def _split_multiwait(nc):
    """This walrus build accepts at most one sem wait per instruction. Hoist
    excess waits onto freshly created same-engine nops placed immediately
    before the over-limit instruction (engine streams run in order, so the
    nop blocking first preserves the dependency)."""
    import bass_rust

    engmap = {
        mybir.EngineType.PE: nc.tensor,
        mybir.EngineType.DVE: nc.vector,
        mybir.EngineType.Activation: nc.scalar,
        mybir.EngineType.SP: nc.sync,
        mybir.EngineType.Pool: nc.gpsimd,
    }
    blocks = list(nc.main_func.blocks)
    records = []
    for bi, bb in enumerate(blocks):
        for ins in bb.instructions:
            si = ins.sync_info
            if si is None or not si.on_wait:
                continue
            waits = list(si.on_wait)
            limit = 2 if type(ins).__name__ == "InstEventSemaphore" else 1
            if len(waits) > limit:
                records.append((ins.name, ins, waits[:-limit]))
                si.on_wait = waits[-limit:]
    if not records:
        return
    carriers = {}
    nop_names = set()
    for name, ins, excess in records:
        lst = []
        for w in excess:
            nb = engmap[ins.engine].nop()
            nb.ins.sync_info = bass_rust.SyncInfo(on_wait=[w], on_update=[])
            nop_names.add(nb.ins.name)
            lst.append(nb.ins)
        carriers[name] = lst
    for bb in blocks:
        il = list(bb.instructions)
        out = []
        changed = False
        for ins in il:
            if ins.name in nop_names:
                changed = True
                continue
            if ins.name in carriers:
                out.extend(carriers[ins.name])
                changed = True
            out.append(ins)
        if changed:
            bb.instructions = out


def _host_constants():
    pos = np.arange(T, dtype=np.float32)[:, None]
    div = np.exp(
        np.arange(0, C, 2, dtype=np.float32) * (-math.log(10000.0) / C)
    )
    ang = pos * div
    pe = np.stack([np.sin(ang), np.cos(ang)], axis=-1).reshape(T, C)
    peT = np.ascontiguousarray(pe.T)                    # [C, T]

    ident = np.eye(128, dtype=np.float32)
    kk = np.arange(128)
    # mask01[k, q] = 0 where q < k (future key within diagonal block)
    mask01 = np.where(kk[None, :] < kk[:, None], 0.0, 1.0).astype(E4)
    return peT, ident, mask01


def _build(flags):
    """flags = (g1, b1ln, g2, b2ln, b1, b2) booleans for non-trivial params."""
    has_g1, has_b1ln, has_g2, has_b2ln, has_b1, has_b2 = flags
    _patch_drain()
    nc = bass.Bass(trn_type="TRN2")

    # ---- DRAM I/O ----
    xq_d = nc.dram_tensor("xq8", [128, KT, T], F8, kind="ExternalInput")
    x_tc = nc.dram_tensor("x_tc", [T, C], BF16, kind="ExternalInput")
    wqk_d = nc.dram_tensor("wqk8", [128, 16, KT, 128], F8, kind="ExternalInput")
    wv_d = nc.dram_tensor("wv8", [128, KT, C], F8, kind="ExternalInput")
    wo_d = nc.dram_tensor("wo8", [128, KT, C], F8, kind="ExternalInput")
    w18_d = nc.dram_tensor("w18", [128, 3, 2, JT, 128], F8,
                           kind="ExternalInput")
    w1t_d = nc.dram_tensor("w1t", [128, 2, JT, 128], BF16,
                           kind="ExternalInput")
    w2_d = nc.dram_tensor("w2b", [128, JT, C], BF16, kind="ExternalInput")
    idb_d = nc.dram_tensor("identb", [128, 128], BF16, kind="ExternalInput")
    mk_d = nc.dram_tensor("mask01", [128, 128], F8, kind="ExternalInput")
    if has_g1:
        g1_d = nc.dram_tensor("g1", [C], F32, kind="ExternalInput")
    if has_b1ln:
        b1ln_d = nc.dram_tensor("b1ln", [C], F32, kind="ExternalInput")
    if has_g2:
        g2_d = nc.dram_tensor("g2", [C], F32, kind="ExternalInput")
    if has_b2ln:
        b2ln_d = nc.dram_tensor("b2ln", [C], F32, kind="ExternalInput")
    if has_b1:
        b1_d = nc.dram_tensor("b1t", [128, JT], F32, kind="ExternalInput")
    if has_b2:
        b2_d = nc.dram_tensor("b2", [C], F32, kind="ExternalInput")
    out_d = nc.dram_tensor("out", [C, T], F32, kind="ExternalOutput")

    def bcast_ap(dram_1d, n):
        return bass.AP(tensor=dram_1d.tensor, offset=0, ap=[[0, 128], [1, n]])

    with tile.TileContext(nc) as tc:
        with (
            tc.tile_pool(name="consts", bufs=1) as consts,
            tc.tile_pool(name="smalls", bufs=12) as smalls,
            tc.tile_pool(name="p_hbf", bufs=1) as p_hbf,
            tc.tile_pool(name="p_hT", bufs=1) as p_hT,
        ):
            # ---- constants (ACT-ring DMAs; SP ring is kept clear for the
            # latency-critical xq8/wqk loads) ----
            zbias = consts.tile([128, 1], F32)
            nc.vector.memset(zbias, 0.0)
            nc.const_aps.aps[(F32, 0.0)] = zbias
            epsb = consts.tile([128, 1], F32)
            nc.vector.memset(epsb, EPS)
            lnb = consts.tile([128, 1], F32)
            nc.vector.memset(lnb, float(math.log(SE)))
            warm = consts.tile([128, 64], BF16)
            nc.vector.memset(warm, 0.25)
            tblw = smalls.tile([128, 1], F32, tag="tblw")
            nc.scalar.activation(tblw, epsb, AF.Exp, bias=0.0, scale=1.0)
            nc.scalar.activation(tblw, epsb, AF.Sqrt, bias=0.0, scale=1.0)
            mask01 = consts.tile([128, 128], F8)
            nc.gpsimd.dma_start(out=mask01, in_=mk_d[:, :])
            identb = consts.tile([128, 128], BF16)
            nc.gpsimd.dma_start(out=identb, in_=idb_d[:, :])
            g1bc = b1lnbc = g2bc = b2lnbc = b1sb = b2bc = None
            if has_g1:
                g1bc = consts.tile([128, C], F32)
                nc.scalar.dma_start(out=g1bc, in_=bcast_ap(g1_d, C))
            if has_b1ln:
                b1lnbc = consts.tile([128, C], F32)
                nc.scalar.dma_start(out=b1lnbc, in_=bcast_ap(b1ln_d, C))
            if has_g2:
                g2bc = consts.tile([128, C], F32)
                nc.scalar.dma_start(out=g2bc, in_=bcast_ap(g2_d, C))
            if has_b2ln:
                b2lnbc = consts.tile([128, C], F32)
                nc.scalar.dma_start(out=b2lnbc, in_=bcast_ap(b2ln_d, C))
            if has_b1:
                b1sb = consts.tile([128, JT], F32)
                nc.scalar.dma_start(out=b1sb, in_=b1_d[:, :])
            if has_b2:
                b2bc = consts.tile([128, C], F32)
                nc.scalar.dma_start(out=b2bc, in_=bcast_ap(b2_d, C))

            # ---- PE warm-up: ~6us of tiny matmuls during the input-DMA
            # wait so the HAM clock-gate reaches K=8/8 before real work ----
            with tc.tile_pool(name="pp_warm", bufs=1, space="PSUM") as pp_w:
                wps = pp_w.tile([64, 64], F32)
                for _ in range(84):
                    nc.tensor.matmul(wps, lhsT=warm[:, 0:64],
                                     rhs=warm[:, 0:64], start=True, stop=True)

            hbf = p_hbf.tile([128, MT, C], BF16)
            hT8 = p_hT.tile([128, 6, T], F8)
            hTb = p_hT.tile([128, 2, T], BF16)

            def layernorm(resid, out_tile, gbc, bbc, zpool):
                stats = smalls.tile([128, 2, 6], F32, tag="stats")
                nc.vector.bn_stats(out=stats[:, 0, :], in_=resid[:, 0:512])
                nc.vector.bn_stats(out=stats[:, 1, :], in_=resid[:, 512:1024])
                mv = smalls.tile([128, 2], F32, tag="mv")
                nc.vector.bn_aggr(out=mv, in_=stats)
                std = smalls.tile([128, 1], F32, tag="std")
                nc.scalar.activation(std, mv[:, 1:2], AF.Sqrt, bias=epsb)
                istd = smalls.tile([128, 1], F32, tag="istd")
                nc.vector.reciprocal(istd, std)
                nbias = smalls.tile([128, 1], F32, tag="nbias")
                nc.vector.tensor_scalar(
                    out=nbias, in0=mv[:, 0:1], scalar1=istd, scalar2=-1.0,
                    op0=OP.mult, op1=OP.mult,
                )
                if gbc is None and bbc is None:
                    # two halves: downstream per-128-col transposes unblock
                    # after half 0 (subtile deps), hiding the chain latency
                    nc.scalar.activation(
                        out_tile[:, 0:512], resid[:, 0:512],
                        AF.Identity, bias=nbias, scale=istd,
                    )
                    nc.scalar.activation(
                        out_tile[:, 512:1024], resid[:, 512:1024],
                        AF.Identity, bias=nbias, scale=istd,
                    )
                else:
                    z = zpool.tile([128, C], F32, tag="zln")
                    nc.vector.tensor_scalar(
                        out=z, in0=resid, scalar1=istd, scalar2=nbias,
                        op0=OP.mult, op1=OP.add,
                    )
                    if gbc is not None:
                        nc.vector.tensor_mul(z, z, gbc)
                    if bbc is not None:
                        nc.vector.tensor_add(z, z, bbc)
                    nc.vector.tensor_copy(out_tile, z)

            # ================= front: QKV + attention =================
            with (
                tc.tile_pool(name="p_ctxT", bufs=1) as p_ctxT,
                tc.tile_pool(name="p_ln1", bufs=3) as p_ln1,
                tc.tile_pool(name="p_w1", bufs=4) as p_w1,
            ):
                ctxT8 = p_ctxT.tile([128, KT, T], F8)
                wo8 = p_ctxT.tile([128, KT, C], F8)
                # xt / w1 pools live OUTSIDE the attention pools so their
                # DMAs have no address-reuse deps and land during attention
                xts = {}

                def xt_dma(m):
                    xt = p_ln1.tile([128, C], BF16, tag="xt",
                                    name=f"xt_{m}")
                    nc.sync.dma_start(
                        out=xt, in_=x_tc[m * 128:(m + 1) * 128, :]
                    )
                    xts[m] = xt

                w1cs = {}

                def w1_dma(jc):
                    w8c = p_w1.tile([128, 3, 2, 4, 128], F8, tag="w8c",
                                    name=f"w8c_{jc}")
                    nc.gpsimd.dma_start(
                        out=w8c, in_=w18_d[:, :, :, jc * 4:(jc + 1) * 4, :]
                    )
                    wtc = p_w1.tile([128, 2, 4, 128], BF16, tag="wtc",
                                    name=f"wtc_{jc}")
                    nc.gpsimd.dma_start(
                        out=wtc, in_=w1t_d[:, :, jc * 4:(jc + 1) * 4, :]
                    )
                    w1cs[jc] = (w8c, wtc)

                with (
                    tc.tile_pool(name="p_att", bufs=1) as p_att,
                    tc.tile_pool(name="p_ex", bufs=4) as p_ex,
                    tc.tile_pool(name="pp_big", bufs=8, space="PSUM") as pp_big,
                ):
                    qkT = p_att.tile([128, 16, T], BF16)
                    vsb = p_att.tile([128, MT, H * (HD + 1)], F8)
                    v4 = vsb.rearrange("p m (h e) -> p m h e", h=H)
                    ctxf = p_att.tile([128, MT, C], BF16)
                    xq8 = p_att.tile([128, KT, T], F8)
                    wqk8 = p_att.tile([128, 16, KT, 128], F8)
                    wv8 = p_att.tile([128, KT, C], F8)

                    nc.sync.dma_start(out=xq8[:, 0:2, :],
                                      in_=xq_d[:, 0:2, :])
                    nc.scalar.dma_start(out=xq8[:, 2:4, :],
                                      in_=xq_d[:, 2:4, :])
                    nc.gpsimd.dma_start(out=xq8[:, 4:6, :],
                                        in_=xq_d[:, 4:6, :])
                    nc.scalar.dma_start(out=xq8[:, 6:8, :],
                                      in_=xq_d[:, 6:8, :])
                    nc.sync.dma_start(out=wqk8[:, 0, :, :],
                                      in_=wqk_d[:, 0, :, :])
                    nc.sync.dma_start(out=wqk8[:, 8, :, :],
                                      in_=wqk_d[:, 8, :, :])
                    for lo, hi in ((1, 3), (9, 11), (3, 6), (11, 14),
                                   (6, 8), (14, 16)):
                        nc.sync.dma_start(out=wqk8[:, lo:hi, :, :],
                                          in_=wqk_d[:, lo:hi, :, :])
                    nc.scalar.dma_start(out=wv8, in_=wv_d[:, :, :])
                    nc.gpsimd.dma_start(out=wo8, in_=wo_d[:, :, :])
                    w1_dma(0)
                    w1_dma(1)
                    w1_dma(2)
                    w1_dma(3)
                    xt_dma(0)
                    xt_dma(1)
                    nc.vector.memset(v4[:, :, :, HD:HD + 1], 1.0)

                    def qkv_unit(m, n):
                        def u():
                            mm = pp_big.tile([128, 512], F32, tag="big")
                            for ks in range(4):
                                nc.tensor.matmul(
                                    mm,
                                    lhsT=wqk8[:, m, 2 * ks:2 * ks + 2, :],
                                    rhs=xq8[
                                        :, 2 * ks:2 * ks + 2,
                                        n * 512:(n + 1) * 512,
                                    ],
                                    start=(ks == 0), stop=(ks == 3),
                                    perf_mode=PM.DoubleRow,
                                )
                            nc.vector.tensor_copy(
                                qkT[:, m, n * 512:(n + 1) * 512], mm
                            )
                        return u

                    def vproj_unit(m, n):
                        def u():
                            mm = pp_big.tile([128, 512], F32, tag="big")
                            for ks in range(4):
                                nc.tensor.matmul(
                                    mm,
                                    lhsT=xq8[
                                        :, 2 * ks:2 * ks + 2,
                                        m * 128:(m + 1) * 128,
                                    ],
                                    rhs=wv8[
                                        :, 2 * ks:2 * ks + 2,
                                        n * 512:(n + 1) * 512,
                                    ],
                                    start=(ks == 0), stop=(ks == 3),
                                    perf_mode=PM.DoubleRow,
                                )
                            nc.vector.tensor_scalar_mul(
                                v4[:, m, n * 8:(n + 1) * 8, 0:HD],
                                mm.rearrange("p (h e) -> p h e", h=8),
                                1.0 / SV,
                            )
                        return u

                    exmap = {}

                    def sc_pair_units(a):
                        """Score+exp units for head pair (2a, 2a+1).  The even
                        head's kt/qt live on partitions 0-63, the odd head's on
                        64-127, so the two back-to-back matmuls land on disjoint
                        PE row groups (tile_position auto-derives) and execute
                        concurrently — 2x score throughput."""
                        he, ho = 2 * a, 2 * a + 1
                        exe = p_ex.tile([128, MT, T], F8, tag="ex",
                                        name=f"ex_{he}")
                        exo = p_ex.tile([128, MT, T], F8, tag="ex",
                                        name=f"ex_{ho}")
                        exmap[he], exmap[ho] = exe, exo
                        units = []
                        for j in range(MT):
                            qspan = T - j * 128
                            for lo in range(0, qspan, 512):
                                hi = min(lo + 512, qspan)

                                def u(j=j, lo=lo, hi=hi):
                                    qlo, qhi = j * 128 + lo, j * 128 + hi
                                    ste = pp_big.tile(
                                        [128, 512], F32, tag="big"
                                    )
                                    sto = pp_big.tile(
                                        [128, 512], F32, tag="big"
                                    )
                                    nc.tensor.matmul(
                                        ste[:, 0:hi - lo],
                                        lhsT=qkT[0:64, 8 + a,
                                                 j * 128:(j + 1) * 128],
                                        rhs=qkT[0:64, a, qlo:qhi],
                                        start=True, stop=True,
                                    )
                                    nc.tensor.matmul(
                                        sto[:, 0:hi - lo],
                                        lhsT=qkT[64:128, 8 + a,
                                                 j * 128:(j + 1) * 128],
                                        rhs=qkT[64:128, a, qlo:qhi],
                                        start=True, stop=True,
                                    )
                                    for exh, st in ((exe, ste), (exo, sto)):
                                        nc.scalar.activation(
                                            exh[:, j, qlo:qhi],
                                            st[:, 0:hi - lo],
                                            AF.Exp, bias=lnb, scale=ESC,
                                        )
                                        if lo == 0:
                                            nc.gpsimd.tensor_mul(
                                                exh[:, j,
                                                    j * 128:(j + 1) * 128],
                                                exh[:, j,
                                                    j * 128:(j + 1) * 128],
                                                mask01,
                                            )
                                units.append(u)
                        return units

                    def ctx_unit(h, i2):
                        exh = exmap[h]

                        def u():
                            # normal-mode fp8: 128-col weight loads keep the
                            # compiler's fast-weight-load (DoubleRow at free
                            # dim 65 is LDWEIGHTS-bound and congests the
                            # weight port for qkv/score loads)
                            cps = pp_big.tile([128, 4, HD + 1], F32,
                                              tag="big", padded_shape=None)
                            for ii in range(4):
                                i = i2 * 4 + ii
                                for j in range(i + 1):
                                    nc.tensor.matmul(
                                        cps[:, ii, :],
                                        lhsT=exh[:, j, i * 128:(i + 1) * 128],
                                        rhs=v4[:, j, h, :],
                                        start=(j == 0), stop=(j == i),
                                    )
                            rden = smalls.tile([128, 4], F32, tag="rden")
                            nc.vector.reciprocal(
                                rden,
                                cps.rearrange("p i e -> p (i e)")[:, HD::HD + 1],
                            )
                            rb = bass.AP(
                                tensor=rden.tensor, offset=rden.offset,
                                ap=[rden.ap[0], [rden.ap[1][0], 4], [0, HD]],
                            )
                            nc.vector.tensor_mul(
                                ctxf[:, i2 * 4:(i2 + 1) * 4,
                                     h * HD:(h + 1) * HD],
                                cps[:, :, 0:HD], rb,
                            )
                        return u

                    def p4_unit(b, n):
                        def u():
                            tr = pp_big.tile([128, 512], BF16, tag="big")
                            for a in range(4):
                                nc.tensor.transpose(
                                    tr[:, a * 128:(a + 1) * 128],
                                    ctxf[:, n * 4 + a, b * 128:(b + 1) * 128],
                                    identb,
                                )
                            nc.vector.tensor_copy(
                                ctxT8[:, b, n * 512:(n + 1) * 512], tr
                            )
                        return u

                    # --- software-pipelined emission: paired score chunks
                    # (2x concurrent via row tiling) are spread uniformly
                    # between other PE work so the exp stream never starves;
                    # qkT pairs are produced one slot ahead. ---
                    for u in (qkv_unit(0, 0), qkv_unit(0, 1),
                              qkv_unit(8, 0), qkv_unit(8, 1)):
                        u()
                    for a in range(8):
                        others = []
                        if a + 1 < 8:
                            others += [qkv_unit(a + 1, 0), qkv_unit(a + 1, 1),
                                       qkv_unit(9 + a, 0), qkv_unit(9 + a, 1)]
                        if a == 0:
                            others += [vproj_unit(m, n)
                                       for m in range(8) for n in range(2)]
                        if a >= 1:
                            hp = 2 * (a - 1)
                            others += [ctx_unit(hp, 0), ctx_unit(hp, 1),
                                       ctx_unit(hp + 1, 0), ctx_unit(hp + 1, 1)]
                        if a >= 2:
                            others += [p4_unit(a - 2, 0), p4_unit(a - 2, 1)]
                        sts = sc_pair_units(a)
                        ns = len(sts)
                        no = len(others)
                        for idx, su in enumerate(sts):
                            su()
                            for u in others[
                                (idx * no) // ns:((idx + 1) * no) // ns
                            ]:
                                u()
                    for u in (ctx_unit(H - 2, 0), ctx_unit(H - 2, 1),
                              ctx_unit(H - 1, 0), ctx_unit(H - 1, 1),
                              p4_unit(6, 0), p4_unit(6, 1),
                              p4_unit(7, 0), p4_unit(7, 1)):
                        u()

                # ---- out-proj + residual + LN1 + h^T + FFN1 overlap ----
                # Residual rides the PE: the out-proj PSUM group starts with
                # identb.T @ (512*x)  (bf16), then accumulates the fp8 DR
                # out-proj matmuls.  LN is scale-invariant, so the 512x
                # scale needs no unscale; bn_stats reads the PSUM directly.
                with (
                    tc.tile_pool(name="p_ff1", bufs=1) as p_ff1,
                    tc.tile_pool(name="p_w2r", bufs=1) as p_w2r,
                ):
                  ff1 = p_ff1.tile([128, JT, T], BF16)
                  w2sb = p_w2r.tile([128, JT, C], BF16)

                  def w2_dma(jc):
                      nc.gpsimd.dma_start(
                          out=w2sb[:, jc * 4:(jc + 1) * 4, :],
                          in_=w2_d[:, jc * 4:(jc + 1) * 4, :],
                      )

                  with (
                    tc.tile_pool(name="pp_ao", bufs=2, space="PSUM") as pp_ao,
                    tc.tile_pool(name="pp_f1", bufs=3, space="PSUM") as pp_f1,
                    tc.tile_pool(name="pp_tr7", bufs=1, space="PSUM") as pp_tr7,
                  ):
                    def op_ln1(m):
                        ao = pp_ao.tile([128, C], F32, tag="ao")
                        for n in range(2):
                            nc.tensor.matmul(
                                ao[:, n * 512:(n + 1) * 512],
                                lhsT=identb,
                                rhs=xts[m][:, n * 512:(n + 1) * 512],
                                start=True, stop=False,
                            )
                            for ks in range(4):
                                nc.tensor.matmul(
                                    ao[:, n * 512:(n + 1) * 512],
                                    lhsT=ctxT8[
                                        :, 2 * ks:2 * ks + 2,
                                        m * 128:(m + 1) * 128,
                                    ],
                                    rhs=wo8[
                                        :, 2 * ks:2 * ks + 2,
                                        n * 512:(n + 1) * 512,
                                    ],
                                    start=False, stop=(ks == 3),
                                    perf_mode=PM.DoubleRow,
                                )
                        layernorm(ao, hbf[:, m, :], g1bc, b1lnbc, p_ln1)

                    def ht_half(n):
                        for b in range(KT):
                            tr = pp_tr7.tile([128, 512], BF16, tag="tr7")
                            for a in range(4):
                                nc.tensor.transpose(
                                    tr[:, a * 128:(a + 1) * 128],
                                    hbf[:, n * 4 + a, b * 128:(b + 1) * 128],
                                    identb,
                                )
                            if b < 6:
                                nc.vector.tensor_scalar_mul(
                                    hT8[:, b, n * 512:(n + 1) * 512], tr, 8.0
                                )
                            else:
                                nc.vector.tensor_scalar_mul(
                                    hTb[:, b - 6, n * 512:(n + 1) * 512],
                                    tr, 16.0,
                                )

                    def f1_half(j, n, dve_relu=False):
                        ps = pp_f1.tile([128, 512], F32, tag="f1h")
                        w8c, wtc = w1cs[j // 4]
                        jj = j % 4
                        for p in range(3):
                            nc.tensor.matmul(
                                ps,
                                lhsT=w8c[:, p, :, jj, :],
                                rhs=hT8[:, 2 * p:2 * p + 2,
                                        n * 512:(n + 1) * 512],
                                start=(p == 0), stop=False,
                                perf_mode=PM.DoubleRow,
                            )
                        for k in range(2):
                            nc.tensor.matmul(
                                ps,
                                lhsT=wtc[:, k, jj, :],
                                rhs=hTb[:, k, n * 512:(n + 1) * 512],
                                start=False, stop=(k == 1),
                            )
                        if dve_relu and not has_b1:
                            # LN1 window: ACT does the LN chain and DVE the
                            # stats there; run these fillers' ReLU on the
                            # idle GpSimd so the f1 PSUM slots recycle
                            # without delaying either chain engine
                            nc.vector.tensor_scalar(
                                out=ff1[:, j, n * 512:(n + 1) * 512],
                                in0=ps, scalar1=1.0 / 256.0, scalar2=0.0,
                                op0=OP.mult, op1=OP.max,
                            )
                            return
                        bias = b1sb[:, j:j + 1] if has_b1 else 0.0
                        nc.scalar.activation(
                            ff1[:, j, n * 512:(n + 1) * 512], ps,
                            AF.Relu, bias=bias, scale=1.0 / 256.0,
                        )

                    for m in range(MT):
                        if m + 2 < MT:
                            xt_dma(m + 2)
                        op_ln1(m)
                        if m == 3:
                            ht_half(0)
                        elif m >= 4:
                            f1_half(m - 4, 0, dve_relu=True)
                    for j in range(4, 16):
                        f1_half(j, 0, dve_relu=True)
                    ht_half(1)
                    w2_dma(0)
                    for jc in range(4):
                        for jj in range(4):
                            f1_half(jc * 4 + jj, 1)
                        w1_dma(4 + jc)
                        w2_dma(1 + jc)
                    for jc in range(4, 8):
                        if jc < 7:
                            w2_dma(jc + 1)
                        for jj in range(4):
                            j = jc * 4 + jj
                            f1_half(j, 0)
                            f1_half(j, 1)

                  # ---- FFN2 + residual + LN2 + out^T ----
                  with (
                    tc.tile_pool(name="p_ln2", bufs=2) as p_ln2,
                    tc.tile_pool(name="p_z2", bufs=2) as p_z2,
                    tc.tile_pool(name="p_out", bufs=4) as p_out,
                    tc.tile_pool(name="pp_f2", bufs=1, space="PSUM") as pp_f2,
                    tc.tile_pool(name="pp_t11", bufs=2, space="PSUM") as pp_t11,
                  ):
                    def ffn2_m(m):
                        ps = pp_f2.tile([128, C], F32, tag=f"f2_{m % 2}",
                                        name=f"f2ps_{m}")
                        for n in range(2):
                            nc.tensor.matmul(
                                ps[:, n * 512:(n + 1) * 512],
                                lhsT=identb,
                                rhs=hbf[:, m, n * 512:(n + 1) * 512],
                                start=True, stop=False,
                            )
                            for j in range(JT):
                                nc.tensor.matmul(
                                    ps[:, n * 512:(n + 1) * 512],
                                    lhsT=ff1[:, j, m * 128:(m + 1) * 128],
                                    rhs=w2sb[:, j, n * 512:(n + 1) * 512],
                                    start=False, stop=(j == JT - 1),
                                )
                        return ps

                    def ln2_m(m, ps):
                        if has_b2:
                            resid2 = p_ln2.tile([128, C], F32, tag="resid2")
                            nc.vector.tensor_add(resid2, ps, b2bc)
                            src = resid2
                        else:
                            src = ps
                        zt = p_z2.tile([128, C], BF16, tag="z2",
                                       name=f"z2_{m}")
                        layernorm(src, zt, g2bc, b2lnbc, p_ln2)
                        return zt

                    def outT(m, zt):
                        # bf16 transposes (final LN2 output; ~0.1% rounding)
                        for b in range(KT):
                            tr = pp_t11.tile([128, 128], BF16, tag="t11")
                            nc.tensor.transpose(
                                tr, zt[:, b * 128:(b + 1) * 128], identb,
                            )
                            osb = p_out.tile([128, 128], F32, tag="osb")
                            nc.vector.tensor_copy(osb, tr)
                            eng = nc.sync if b % 2 == 0 else nc.scalar
                            eng.dma_start(
                                out=out_d[
                                    b * 128:(b + 1) * 128,
                                    m * 128:(m + 1) * 128,
                                ],
                                in_=osb,
                            )

                    for m in range(MT):
                        ps = ffn2_m(m)
                        outT(m, ln2_m(m, ps))
    _split_multiwait(nc)
    return nc


_prog_cache = {}


def prepare(
    x, in_proj_w, out_proj_w, ln1_g, ln1_b, ln2_g, ln2_b, w1, b1, w2, b2
):
    """Returns (nc, in_maps): the built program plus per-core input maps."""
    x = np.asarray(x, dtype=np.float32)
    in_proj_w = np.asarray(in_proj_w, dtype=np.float32)
    out_proj_w = np.asarray(out_proj_w, dtype=np.float32)
    ln1_g = np.asarray(ln1_g, dtype=np.float32)
    ln1_b = np.asarray(ln1_b, dtype=np.float32)
    ln2_g = np.asarray(ln2_g, dtype=np.float32)
    ln2_b = np.asarray(ln2_b, dtype=np.float32)
    w1 = np.asarray(w1, dtype=np.float32)
    b1 = np.asarray(b1, dtype=np.float32)
    w2 = np.asarray(w2, dtype=np.float32)
    b2 = np.asarray(b2, dtype=np.float32)

    flags = (
        not np.all(ln1_g == 1.0),
        not np.all(ln1_b == 0.0),
        not np.all(ln2_g == 1.0),
        not np.all(ln2_b == 0.0),
        not np.all(b1 == 0.0),
        not np.all(b2 == 0.0),
    )
    if flags not in _prog_cache:
        _prog_cache[flags] = _build(flags)
    nc = _prog_cache[flags]

    peT, ident, mask01 = _host_constants()

    winT = np.ascontiguousarray(in_proj_w.T)             # [C, 3C] f32
    wqk = np.ascontiguousarray(np.transpose(
        (winT[:, :2 * C] * SW).reshape(KT, 128, 16, 128), (1, 2, 0, 3)
    )).astype(E4)                                        # [128, 16, KT, 128]
    wv = np.ascontiguousarray(np.transpose(
        (winT[:, 2 * C:] * SW).reshape(KT, 128, C), (1, 0, 2)
    )).astype(E4)                                        # [128, KT, C]
    wo = np.ascontiguousarray(np.transpose(
        (out_proj_w.T * SWO).reshape(KT, 128, C), (1, 0, 2)
    )).astype(E4)                                        # [128, KT, C]
    w1r = np.transpose(
        w1.T.reshape(KT, 128, JT, 128), (1, 0, 2, 3)
    )                                                    # [128, KT, JT, 128]
    w18 = np.ascontiguousarray(
        (w1r[:, 0:6] * 32.0).reshape(128, 3, 2, JT, 128)
    ).astype(E4)                                         # [128, 3, 2, JT, 128]
    w1t = np.ascontiguousarray(w1r[:, 6:8] * 16.0).astype(BF)
    w2b = np.ascontiguousarray(np.transpose(
        w2.T.reshape(JT, 128, C), (1, 0, 2)
    )).astype(BF)                                        # [128, JT, C]

    shared = {
        "wqk8": wqk,
        "wv8": wv,
        "wo8": wo,
        "w18": w18,
        "w1t": w1t,
        "w2b": w2b,
        "identf": ident.astype(np.float32),
        "identb": ident.astype(BF),
        "mask01": mask01,
    }
    if flags[0]:
        shared["g1"] = ln1_g
    if flags[1]:
        shared["b1ln"] = ln1_b
    if flags[2]:
        shared["g2"] = ln2_g
    if flags[3]:
        shared["b2ln"] = ln2_b
    if flags[4]:
        shared["b1t"] = np.ascontiguousarray(b1.reshape(JT, 128).T)
    if flags[5]:
        shared["b2"] = b2

    in_maps = []
    for i in range(N):
        m = dict(shared)
        xq = (x[i] + peT) * SX                            # [C, T]
        m["xq8"] = np.ascontiguousarray(
            np.transpose(xq.reshape(KT, 128, T), (1, 0, 2))
        ).astype(E4)                                      # [128, KT, T]
        # residual ships pre-scaled by SV*SWO (the attention-psum scale;
        # LN is scale-invariant) so it can ride the out-proj matmul group
        m["x_tc"] = np.ascontiguousarray(
            x[i].T * (SV * SWO)
        ).astype(BF)                                      # [T, C] bf16
        in_maps.append(m)
    return nc, in_maps


def kernel(**inputs):
    nc, in_maps = prepare(**inputs)
    res = run_bass_kernel_spmd(nc, in_maps, core_ids=list(range(N)))
    out = np.stack([res.results[i]["out"] for i in range(N)], axis=0)
    return out.astype(np.float32)

